# revision 4
# baseline (speedup 1.0000x reference)
"""Trainium2 kernel for nn_Decoder_featurizer: HRR decoder featurization.

reference: out = concat([p, l, assoc(dec_d, p)..., assoc(dec_d, l)...], -1)
where assoc(d, x)[j] = sum_t d[t] * x[(j+t) % N]  (circular correlation).

Circular correlation is a circulant matmul y = x @ C_d with
C_d[k, j] = dec[d, (k-j) % N].  v3 cuts the PE work 2.67x below the dense
circulant via the CRT factorization of x^1024 - 1:

  x^1024-1 = (x^512-1)(x^512+1);  x^512-1 = (x^256-1)(x^256+1)

With x+ = x[:512]+x[512:], x- = x[:512]-x[512:], x++ = x+[:256]+x+[256:],
x+- = x+[:256]-x+[256:] (host-precomputed, shipped transposed in bf16), the
device computes per decoder only three small matmuls:

  Pm = x-  @ skew512(h_d)/2    [B,512]   h  = d[:512]-d[512:]
  Qp = x++ @ circ256(gp_d)/4   [B,256]   g  = d[:512]+d[512:]
  Qm = x+- @ skew256(gm_d)/4   [B,256]   gp = g[:256]+g[256:], gm = g[:256]-g[256:]

(3072 PE cycles per 128-row tile per decoder vs 8192 dense).  Device output
is the raw [Pm|Qp|Qm] partials in bf16 (half the HBM write traffic); the
host does the butterfly recombine  t=[Qp+Qm,Qp-Qm]; y=[t+Pm,t-Pm]  and
assembles the full f32 result (passthrough columns come straight from the
inputs, never touching the device).

All weights and activations are SBUF-resident (skew/circulant blocks live
in extended slot buffers so any k-tile's weight row-block is a contiguous
slice).  Per 128-row m-tile: 4 groups of 4 decoders, each group = phase A
(4 psum banks of Pm) + phase B (4 banks of [Qp|Qm]); DVE drains A banks
while PE runs B, ScalarE drains B while PE runs the next group's A.
Redundant LDWEIGHTS (4 MMs share each stationary x-tile) are deduped at
the BIR level.
"""

import numpy as np
import ml_dtypes

HRR = 1024
D = 16
B = 8192
NCORES = 8
BPC = B // NCORES            # batch rows per core
ROWS = 2 * BPC               # rows per core (problem + lemma stacked)
DN = D * HRR                 # 16384 assoc features per input
OUT_COLS = 2 * HRR + 2 * DN  # 34816

_CACHE = {}


def _build_program(loop_iters: int = 1):
    import contextlib
    import concourse.bacc as bacc
    import concourse.mybir as mybir
    from concourse.tile import TileContext

    nc = bacc.Bacc("TRN2", target_bir_lowering=False, debug=False,
                   num_devices=NCORES)
    xmT = nc.dram_tensor("xmT", [512, ROWS], mybir.dt.bfloat16,
                         kind="ExternalInput").ap()
    xppT = nc.dram_tensor("xppT", [256, ROWS], mybir.dt.bfloat16,
                          kind="ExternalInput").ap()
    xpmT = nc.dram_tensor("xpmT", [256, ROWS], mybir.dt.bfloat16,
                          kind="ExternalInput").ap()
    wm = nc.dram_tensor("wm", [128, D * 896], mybir.dt.bfloat16,
                        kind="ExternalInput").ap()
    wpp = nc.dram_tensor("wpp", [128, D * 384], mybir.dt.bfloat16,
                         kind="ExternalInput").ap()
    wpm = nc.dram_tensor("wpm", [128, D * 384], mybir.dt.bfloat16,
                         kind="ExternalInput").ap()
    out = nc.dram_tensor("out", [ROWS, DN], mybir.dt.bfloat16,
                         kind="ExternalOutput").ap()

    with TileContext(nc) as tc:
        with (
            tc.tile_pool(name="xp", bufs=1) as xpool,
            tc.tile_pool(name="wp", bufs=1) as wpool,
            tc.tile_pool(name="ps", bufs=1, space="PSUM") as pspool,
            tc.tile_pool(name="ob", bufs=2) as opool,
        ):
            # resident transposed activations
            xm = []
            for k in range(4):
                t = xpool.tile([128, ROWS], mybir.dt.bfloat16, tag=f"xm{k}")
                nc.sync.dma_start(out=t[:], in_=xmT[k * 128:(k + 1) * 128, :])
                xm.append(t)
            xpp = []
            for k in range(2):
                t = xpool.tile([128, ROWS], mybir.dt.bfloat16, tag=f"xpp{k}")
                nc.sync.dma_start(out=t[:], in_=xppT[k * 128:(k + 1) * 128, :])
                xpp.append(t)
            xpm = []
            for k in range(2):
                t = xpool.tile([128, ROWS], mybir.dt.bfloat16, tag=f"xpm{k}")
                nc.sync.dma_start(out=t[:], in_=xpmT[k * 128:(k + 1) * 128, :])
                xpm.append(t)

            # resident extended weight slot buffers
            wmt = wpool.tile([128, D * 896], mybir.dt.bfloat16, tag="wm")
            nc.sync.dma_start(out=wmt[:], in_=wm[:, :])
            wppt = wpool.tile([128, D * 384], mybir.dt.bfloat16, tag="wpp")
            nc.sync.dma_start(out=wppt[:], in_=wpp[:, :])
            wpmt = wpool.tile([128, D * 384], mybir.dt.bfloat16, tag="wpm")
            nc.sync.dma_start(out=wpmt[:], in_=wpm[:, :])

            loop_cm = (tc.For_i(0, loop_iters, 1,
                                hint_engines=(mybir.EngineType.PE,
                                              mybir.EngineType.SP,
                                              mybir.EngineType.DVE,
                                              mybir.EngineType.Activation))
                       if loop_iters > 1 else contextlib.nullcontext())
            with loop_cm:
              for m in range(16):
                stage = opool.tile([128, DN], mybir.dt.bfloat16, name="stage")
                ms = slice(m * 128, (m + 1) * 128)
                for grp in range(4):
                    d0 = grp * 4
                    # phase A: Pm for 4 decoders (4 psum banks)
                    psA = pspool.tile([128, 2048], mybir.dt.float32,
                                      name="psA")
                    for k in range(4):
                        lhsT = xm[k][:, ms]
                        s0 = (3 - k) * 128
                        for i in range(4):
                            w0 = (d0 + i) * 896 + s0
                            nc.tensor.matmul(
                                psA[:, i * 512:(i + 1) * 512], lhsT,
                                wmt[:, w0:w0 + 512],
                                start=(k == 0), stop=(k == 3))
                    # phase B: [Qp|Qm] for 4 decoders (4 psum banks)
                    psB = pspool.tile([128, 2048], mybir.dt.float32,
                                      name="psB")
                    for k in range(2):
                        lhsT = xpp[k][:, ms]
                        s0 = (1 - k) * 128
                        for i in range(4):
                            w0 = (d0 + i) * 384 + s0
                            nc.tensor.matmul(
                                psB[:, i * 512:i * 512 + 256], lhsT,
                                wppt[:, w0:w0 + 256],
                                start=(k == 0), stop=(k == 1))
                    for k in range(2):
                        lhsT = xpm[k][:, ms]
                        s0 = (1 - k) * 128
                        for i in range(4):
                            w0 = (d0 + i) * 384 + s0
                            nc.tensor.matmul(
                                psB[:, i * 512 + 256:(i + 1) * 512], lhsT,
                                wpmt[:, w0:w0 + 256],
                                start=(k == 0), stop=(k == 1))
                    # drains: A banks must free during phase B (psA is
                    # reused by the next group's phase A), so split A
                    # across DVE+ACT; B banks drain during the next
                    # group's phase A on ACT.
                    for i in range(4):
                        c0 = (d0 + i) * 1024
                        eng = nc.vector.tensor_copy if i < 3 else \
                            nc.scalar.copy
                        eng(out=stage[:, c0:c0 + 512],
                            in_=psA[:, i * 512:(i + 1) * 512])
                    for i in range(4):
                        c0 = (d0 + i) * 1024
                        nc.scalar.copy(
                            out=stage[:, c0 + 512:c0 + 1024],
                            in_=psB[:, i * 512:(i + 1) * 512])
                nc.sync.dma_start(out=out[ms, :], in_=stage[:])
    _finalize_with_dedup(nc)
    return nc


def _dedup_ldweights(nc):
    """Drop redundant InstLdweights from the PE stream.

    bacc emits every matmul as an (InstLdweights, InstMatmult) pair; the
    matmult is non-self-loading, so the PE weight register persists across
    matmuls.  Consecutive pairs with an identical stationary AP reload the
    same weights (~107ns each on HW).  Drop an InstLdweights when its
    signature matches the previous one on the PE stream AND it carries no
    waits/updates.  Conservatively resets tracking at block boundaries and
    on any other PE instruction.
    """
    import concourse.mybir as mybir

    InstLdweights = mybir.InstLdweights
    InstMatmult = mybir.InstMatmult
    n_drop = 0
    for fn in nc.m.functions:
        for blk in fn.blocks:
            keep = []
            last_sig = None
            for inst in blk.instructions:
                if isinstance(inst, InstLdweights):
                    pap = inst.ins[0]
                    sig = (pap.memref, pap.offset, str(pap.ap),
                           str(pap.dtype),
                           str(getattr(inst, "perf_mode", None)),
                           str(getattr(inst, "is_transpose", None)),
                           str(getattr(inst, "tile_position", None)))
                    si = inst.sync_info
                    bare = si is None or (len(si.on_wait) == 0
                                          and len(si.on_update) == 0)
                    if sig == last_sig and bare:
                        n_drop += 1
                        continue
                    last_sig = sig
                elif getattr(inst, "engine", None) == mybir.EngineType.PE:
                    if isinstance(inst, InstMatmult):
                        if getattr(inst, "is_transpose", None):
                            last_sig = None
                    else:
                        last_sig = None
                keep.append(inst)
            if n_drop:
                try:
                    blk.instructions = keep
                except Exception:
                    insts = blk.instructions
                    while len(insts):
                        insts.pop()
                    for i in keep:
                        insts.append(i)
    return n_drop


def _finalize_with_dedup(nc):
    orig_mv = nc.move_matmul_waits_to_ldweights

    def _mv():
        orig_mv()
        _dedup_ldweights(nc)

    nc.move_matmul_waits_to_ldweights = _mv
    nc.finalize()


def _get_program(loop_iters: int = 1):
    key = f"nc{loop_iters}"
    if key not in _CACHE:
        _CACHE[key] = _build_program(loop_iters)
    return _CACHE[key]


def _ext_buffer(f, T):
    """Extended slot buffer [128, (2T-1)*128]; slot s holds block(T-1-s),
    block(r)[p, q] = f(r*128 + p - q).  For k-tile k of the [T*128, T*128]
    structured matrix, rows k*128:(k+1)*128 (all columns) are the
    contiguous slice starting at slot (T-1-k)."""
    p = np.arange(128)[:, None]
    q = np.arange(128)[None, :]
    out = np.empty((128, (2 * T - 1) * 128), np.float32)
    for s in range(2 * T - 1):
        r = (T - 1) - s
        out[:, s * 128:(s + 1) * 128] = f(r * 128 + p - q)
    return out


def _build_weights(decoders: np.ndarray):
    """Per-decoder extended slot buffers for skew512(h)/2, circ256(gp)/4,
    skew256(gm)/4, concatenated decoder-major, bf16."""
    dec = np.asarray(decoders, np.float32)
    wm = np.empty((128, D * 896), np.float32)
    wpp = np.empty((128, D * 384), np.float32)
    wpm = np.empty((128, D * 384), np.float32)
    for d in range(D):
        v = dec[d]
        g = v[:512] + v[512:]
        h = (v[:512] - v[512:]) / 2
        gp = (g[:256] + g[256:]) / 4
        gm = (g[:256] - g[256:]) / 4

        def f_skew(i, vv, M):
            i = np.asarray(i)
            return np.where(i >= 0, vv[i % M], -vv[i % M])

        wm[:, d * 896:(d + 1) * 896] = _ext_buffer(
            lambda i: f_skew(i, h, 512), 4)
        wpp[:, d * 384:(d + 1) * 384] = _ext_buffer(
            lambda i: gp[np.asarray(i) % 256], 2)
        wpm[:, d * 384:(d + 1) * 384] = _ext_buffer(
            lambda i: f_skew(i, gm, 256), 2)
    b = ml_dtypes.bfloat16
    return wm.astype(b), wpp.astype(b), wpm.astype(b)


def _build_in_maps(problemhrr, lemmahrr, decoders):
    b = ml_dtypes.bfloat16
    wm, wpp, wpm = _build_weights(decoders)
    in_maps = []
    for c in range(NCORES):
        p = problemhrr[c * BPC:(c + 1) * BPC]
        l = lemmahrr[c * BPC:(c + 1) * BPC]
        X = np.concatenate([p, l], axis=0)          # [2048, 1024] f32
        xm = X[:, :512] - X[:, 512:]
        xp = X[:, :512] + X[:, 512:]
        xpp = xp[:, :256] + xp[:, 256:]
        xpm = xp[:, :256] - xp[:, 256:]
        in_maps.append({
            "xmT": np.ascontiguousarray(xm.T).astype(b),
            "xppT": np.ascontiguousarray(xpp.T).astype(b),
            "xpmT": np.ascontiguousarray(xpm.T).astype(b),
            "wm": wm, "wpp": wpp, "wpm": wpm,
        })
    return in_maps


def _bf16_to_f32(a: np.ndarray) -> np.ndarray:
    return (a.view(np.uint16).astype(np.uint32) << 16).view(np.float32)


def _recombine(dev_out: np.ndarray) -> np.ndarray:
    """[ROWS, D*1024] bf16 [Pm|Qp|Qm] partials -> [ROWS, D*1024] f32 assoc."""
    arr = _bf16_to_f32(dev_out).reshape(ROWS, D, 1024)
    Pm = arr[:, :, :512]
    Qp = arr[:, :, 512:768]
    Qm = arr[:, :, 768:]
    y = np.empty((ROWS, D, 1024), np.float32)
    t_lo = y[:, :, 0:256]
    np.add(Qp, Qm, out=t_lo)
    t_hi = y[:, :, 256:512]
    np.subtract(Qp, Qm, out=t_hi)
    t = y[:, :, 0:512]
    np.subtract(t, Pm, out=y[:, :, 512:1024])
    np.add(t, Pm, out=t)
    return y.reshape(ROWS, D * 1024)


def kernel(problemhrr: np.ndarray, lemmahrr: np.ndarray,
           decoders: np.ndarray) -> np.ndarray:
    from concourse.bass_utils import run_bass_kernel_spmd

    problemhrr = np.asarray(problemhrr, dtype=np.float32)
    lemmahrr = np.asarray(lemmahrr, dtype=np.float32)
    decoders = np.asarray(decoders, dtype=np.float32)

    nc = _get_program()
    in_maps = _build_in_maps(problemhrr, lemmahrr, decoders)
    res = run_bass_kernel_spmd(nc, in_maps, list(range(NCORES)))

    full = np.empty((B, OUT_COLS), np.float32)
    full[:, :HRR] = problemhrr
    full[:, HRR:2 * HRR] = lemmahrr
    for c in range(NCORES):
        y = _recombine(res.results[c]["out"])
        rows = slice(c * BPC, (c + 1) * BPC)
        full[rows, 2 * HRR:2 * HRR + DN] = y[:BPC]
        full[rows, 2 * HRR + DN:] = y[BPC:]
    return full


# revision 10
# speedup vs baseline: 1.0485x; 1.0485x over previous
"""Trainium2 kernel for nn_Decoder_featurizer: HRR decoder featurization.

reference: out = concat([p, l, assoc(dec_d, p)..., assoc(dec_d, l)...], -1)
where assoc(d, x)[j] = sum_t d[t] * x[(j+t) % N]  (circular correlation).

Circular correlation is a circulant matmul y = x @ C_d with
C_d[k, j] = dec[d, (k-j) % N].  v3 cuts the PE work 2.67x below the dense
circulant via the CRT factorization of x^1024 - 1:

  x^1024-1 = (x^512-1)(x^512+1);  x^512-1 = (x^256-1)(x^256+1)

  x^1024-1 = (x^512-1)(x^512+1);  x^512-1 = (x^256-1)(x^256+1);
  x^256-1 = (x^128-1)(x^128+1)

With x+ = x[:512]+x[512:], x- = x[:512]-x[512:], x++ = x+[:256]+x+[256:],
x+- = x+[:256]-x+[256:], x+++ = x++[:128]+x++[128:], x++- = x++[:128]-
x++[128:] (host-precomputed, shipped transposed in bf16), the device
computes per decoder only four small matmuls:

  Pm  = x-   @ skew512(h_d)/2     [B,512]   h   = d[:512]-d[512:]
  Qm  = x+-  @ skew256(gm_d)/4    [B,256]   g   = d[:512]+d[512:]
  Qpp = x+++ @ circ128(gpp_d)/8   [B,128]   gp  = g[:256]+g[256:]
  Qpm = x++- @ skew128(gpm_d)/8   [B,128]   gm  = g[:256]-g[256:]
                                            gpp = gp[:128]+gp[128:]
                                            gpm = gp[:128]-gp[128:]

(2816 PE cycles per 128-row tile per decoder vs 8192 dense).  Device output
is the raw [Pm|Qm|Qpp|Qpm] partials in bf16 (half the HBM write traffic);
the host does the butterfly recombine
  Qp = [Qpp+Qpm, Qpp-Qpm]; t = [Qp+Qm, Qp-Qm]; y = [t+Pm, t-Pm]
and assembles the full f32 result (passthrough columns come straight from
the inputs, never touching the device).

All weights and activations are SBUF-resident (skew/circulant blocks live
in extended slot buffers so any k-tile's weight row-block is a contiguous
slice).  Per 128-row m-tile: 4 groups of 4 decoders, each group = phase A
(4 psum banks of Pm) + phase B (4 banks of [Qp|Qm]); DVE drains A banks
while PE runs B, ScalarE drains B while PE runs the next group's A.
Redundant LDWEIGHTS (4 MMs share each stationary x-tile) are deduped at
the BIR level.
"""

import numpy as np
import ml_dtypes

HRR = 1024
D = 16
B = 8192
NCORES = 8
BPC = B // NCORES            # batch rows per core
ROWS = 2 * BPC               # rows per core (problem + lemma stacked)
DN = D * HRR                 # 16384 assoc features per input
OUT_COLS = 2 * HRR + 2 * DN  # 34816

_CACHE = {}


def _build_program(loop_iters: int = 1):
    import contextlib
    import concourse.bacc as bacc
    import concourse.mybir as mybir
    from concourse.tile import TileContext

    nc = bacc.Bacc("TRN2", target_bir_lowering=False, debug=False,
                   num_devices=NCORES)
    xmT = nc.dram_tensor("xmT", [512, ROWS], mybir.dt.bfloat16,
                         kind="ExternalInput").ap()
    xpmT = nc.dram_tensor("xpmT", [256, ROWS], mybir.dt.bfloat16,
                          kind="ExternalInput").ap()
    xpppT = nc.dram_tensor("xpppT", [128, ROWS], mybir.dt.bfloat16,
                           kind="ExternalInput").ap()
    xppmT = nc.dram_tensor("xppmT", [128, ROWS], mybir.dt.bfloat16,
                           kind="ExternalInput").ap()
    wm = nc.dram_tensor("wm", [128, D * 896], mybir.dt.bfloat16,
                        kind="ExternalInput").ap()
    wpm = nc.dram_tensor("wpm", [128, D * 384], mybir.dt.bfloat16,
                         kind="ExternalInput").ap()
    wppp = nc.dram_tensor("wppp", [128, D * 128], mybir.dt.bfloat16,
                          kind="ExternalInput").ap()
    wppm = nc.dram_tensor("wppm", [128, D * 128], mybir.dt.bfloat16,
                          kind="ExternalInput").ap()
    out = nc.dram_tensor("out", [ROWS, DN], mybir.dt.bfloat16,
                         kind="ExternalOutput").ap()

    with TileContext(nc) as tc:
        with (
            tc.tile_pool(name="xp", bufs=1) as xpool,
            tc.tile_pool(name="wp", bufs=1) as wpool,
            tc.tile_pool(name="ps", bufs=1, space="PSUM") as pspool,
            tc.tile_pool(name="ob", bufs=2) as opool,
        ):
            # resident transposed activations
            xm = []
            for k in range(4):
                t = xpool.tile([128, ROWS], mybir.dt.bfloat16, tag=f"xm{k}")
                nc.sync.dma_start(out=t[:], in_=xmT[k * 128:(k + 1) * 128, :])
                xm.append(t)
            xpm = []
            for k in range(2):
                t = xpool.tile([128, ROWS], mybir.dt.bfloat16, tag=f"xpm{k}")
                nc.sync.dma_start(out=t[:], in_=xpmT[k * 128:(k + 1) * 128, :])
                xpm.append(t)
            xppp = xpool.tile([128, ROWS], mybir.dt.bfloat16, tag="xppp")
            nc.sync.dma_start(out=xppp[:], in_=xpppT[:, :])
            xppm = xpool.tile([128, ROWS], mybir.dt.bfloat16, tag="xppm")
            nc.sync.dma_start(out=xppm[:], in_=xppmT[:, :])

            # resident extended weight slot buffers
            wmt = wpool.tile([128, D * 896], mybir.dt.bfloat16, tag="wm")
            nc.sync.dma_start(out=wmt[:], in_=wm[:, :])
            wpmt = wpool.tile([128, D * 384], mybir.dt.bfloat16, tag="wpm")
            nc.sync.dma_start(out=wpmt[:], in_=wpm[:, :])
            wpppt = wpool.tile([128, D * 128], mybir.dt.bfloat16, tag="wppp")
            nc.sync.dma_start(out=wpppt[:], in_=wppp[:, :])
            wppmt = wpool.tile([128, D * 128], mybir.dt.bfloat16, tag="wppm")
            nc.sync.dma_start(out=wppmt[:], in_=wppm[:, :])

            loop_cm = (tc.For_i(0, loop_iters, 1,
                                hint_engines=(mybir.EngineType.PE,
                                              mybir.EngineType.SP,
                                              mybir.EngineType.DVE,
                                              mybir.EngineType.Activation))
                       if loop_iters > 1 else contextlib.nullcontext())
            with loop_cm:
              for m in range(16):
                stage = opool.tile([128, DN], mybir.dt.bfloat16, name="stage")
                ms = slice(m * 128, (m + 1) * 128)
                for grp in range(4):
                    d0 = grp * 4
                    # phase A: Pm for 4 decoders (4 psum banks)
                    psA = pspool.tile([128, 2048], mybir.dt.float32,
                                      name="psA")
                    for k in range(4):
                        lhsT = xm[k][:, ms]
                        s0 = (3 - k) * 128
                        for i in range(4):
                            w0 = (d0 + i) * 896 + s0
                            nc.tensor.matmul(
                                psA[:, i * 512:(i + 1) * 512], lhsT,
                                wmt[:, w0:w0 + 512],
                                start=(k == 0), stop=(k == 3))
                    # phase B: [Qm|Qpp|Qpm] for 4 decoders (4 psum banks)
                    psB = pspool.tile([128, 2048], mybir.dt.float32,
                                      name="psB")
                    for k in range(2):
                        lhsT = xpm[k][:, ms]
                        s0 = (1 - k) * 128
                        for i in range(4):
                            w0 = (d0 + i) * 384 + s0
                            nc.tensor.matmul(
                                psB[:, i * 512:i * 512 + 256], lhsT,
                                wpmt[:, w0:w0 + 256],
                                start=(k == 0), stop=(k == 1))
                    lhsT = xppp[:, ms]
                    for i in range(4):
                        w0 = (d0 + i) * 128
                        nc.tensor.matmul(
                            psB[:, i * 512 + 256:i * 512 + 384], lhsT,
                            wpppt[:, w0:w0 + 128], start=True, stop=True)
                    lhsT = xppm[:, ms]
                    for i in range(4):
                        w0 = (d0 + i) * 128
                        nc.tensor.matmul(
                            psB[:, i * 512 + 384:(i + 1) * 512], lhsT,
                            wppmt[:, w0:w0 + 128], start=True, stop=True)
                    # drains: A banks must free during phase B (psA is
                    # reused by the next group's phase A), so split A
                    # across DVE+ACT; B banks drain during the next
                    # group's phase A on ACT.
                    for i in range(4):
                        c0 = (d0 + i) * 1024
                        eng = nc.vector.tensor_copy if i < 3 else \
                            nc.scalar.copy
                        eng(out=stage[:, c0:c0 + 512],
                            in_=psA[:, i * 512:(i + 1) * 512])
                    for i in range(4):
                        c0 = (d0 + i) * 1024
                        nc.scalar.copy(
                            out=stage[:, c0 + 512:c0 + 1024],
                            in_=psB[:, i * 512:(i + 1) * 512])
                nc.sync.dma_start(out=out[ms, :], in_=stage[:])
    _finalize_with_dedup(nc)
    return nc


def _dedup_ldweights(nc):
    """Drop redundant InstLdweights from the PE stream.

    bacc emits every matmul as an (InstLdweights, InstMatmult) pair; the
    matmult is non-self-loading, so the PE weight register persists across
    matmuls.  Consecutive pairs with an identical stationary AP reload the
    same weights (~107ns each on HW).  Drop an InstLdweights when its
    signature matches the previous one on the PE stream AND it carries no
    waits/updates.  Conservatively resets tracking at block boundaries and
    on any other PE instruction.
    """
    import concourse.mybir as mybir

    InstLdweights = mybir.InstLdweights
    InstMatmult = mybir.InstMatmult
    n_drop = 0
    for fn in nc.m.functions:
        for blk in fn.blocks:
            keep = []
            last_sig = None
            for inst in blk.instructions:
                if isinstance(inst, InstLdweights):
                    pap = inst.ins[0]
                    sig = (pap.memref, pap.offset, str(pap.ap),
                           str(pap.dtype),
                           str(getattr(inst, "perf_mode", None)),
                           str(getattr(inst, "is_transpose", None)),
                           str(getattr(inst, "tile_position", None)))
                    si = inst.sync_info
                    bare = si is None or (len(si.on_wait) == 0
                                          and len(si.on_update) == 0)
                    if sig == last_sig and bare:
                        n_drop += 1
                        continue
                    last_sig = sig
                elif getattr(inst, "engine", None) == mybir.EngineType.PE:
                    if isinstance(inst, InstMatmult):
                        if getattr(inst, "is_transpose", None):
                            last_sig = None
                    else:
                        last_sig = None
                keep.append(inst)
            if n_drop:
                try:
                    blk.instructions = keep
                except Exception:
                    insts = blk.instructions
                    while len(insts):
                        insts.pop()
                    for i in keep:
                        insts.append(i)
    return n_drop


def _finalize_with_dedup(nc):
    orig_mv = nc.move_matmul_waits_to_ldweights

    def _mv():
        orig_mv()
        _dedup_ldweights(nc)

    nc.move_matmul_waits_to_ldweights = _mv
    nc.finalize()


def _get_program(loop_iters: int = 1):
    key = f"nc{loop_iters}"
    if key not in _CACHE:
        _CACHE[key] = _build_program(loop_iters)
    return _CACHE[key]


def _ext_buffer(f, T):
    """Extended slot buffer [128, (2T-1)*128]; slot s holds block(T-1-s),
    block(r)[p, q] = f(r*128 + p - q).  For k-tile k of the [T*128, T*128]
    structured matrix, rows k*128:(k+1)*128 (all columns) are the
    contiguous slice starting at slot (T-1-k)."""
    p = np.arange(128)[:, None]
    q = np.arange(128)[None, :]
    out = np.empty((128, (2 * T - 1) * 128), np.float32)
    for s in range(2 * T - 1):
        r = (T - 1) - s
        out[:, s * 128:(s + 1) * 128] = f(r * 128 + p - q)
    return out


def _build_weights(decoders: np.ndarray):
    """Per-decoder extended slot buffers for skew512(h)/2, skew256(gm)/4,
    circ128(gpp)/8, skew128(gpm)/8, concatenated decoder-major, bf16."""
    dec = np.asarray(decoders, np.float32)
    wm = np.empty((128, D * 896), np.float32)
    wpm = np.empty((128, D * 384), np.float32)
    wppp = np.empty((128, D * 128), np.float32)
    wppm = np.empty((128, D * 128), np.float32)
    for d in range(D):
        v = dec[d]
        g = v[:512] + v[512:]
        h = (v[:512] - v[512:]) / 2
        gp = g[:256] + g[256:]
        gm = (g[:256] - g[256:]) / 4
        gpp = (gp[:128] + gp[128:]) / 8
        gpm = (gp[:128] - gp[128:]) / 8

        def f_skew(i, vv, M):
            i = np.asarray(i)
            return np.where(i >= 0, vv[i % M], -vv[i % M])

        wm[:, d * 896:(d + 1) * 896] = _ext_buffer(
            lambda i: f_skew(i, h, 512), 4)
        wpm[:, d * 384:(d + 1) * 384] = _ext_buffer(
            lambda i: f_skew(i, gm, 256), 2)
        wppp[:, d * 128:(d + 1) * 128] = _ext_buffer(
            lambda i: gpp[np.asarray(i) % 128], 1)
        wppm[:, d * 128:(d + 1) * 128] = _ext_buffer(
            lambda i: f_skew(i, gpm, 128), 1)
    b = ml_dtypes.bfloat16
    return wm.astype(b), wpm.astype(b), wppp.astype(b), wppm.astype(b)


def _build_in_maps(problemhrr, lemmahrr, decoders):
    b = ml_dtypes.bfloat16
    wm, wpm, wppp, wppm = _build_weights(decoders)
    in_maps = []
    for c in range(NCORES):
        p = problemhrr[c * BPC:(c + 1) * BPC]
        l = lemmahrr[c * BPC:(c + 1) * BPC]
        X = np.concatenate([p, l], axis=0)          # [2048, 1024] f32
        xm = X[:, :512] - X[:, 512:]
        xp = X[:, :512] + X[:, 512:]
        xpp = xp[:, :256] + xp[:, 256:]
        xpm = xp[:, :256] - xp[:, 256:]
        xppp = xpp[:, :128] + xpp[:, 128:]
        xppm = xpp[:, :128] - xpp[:, 128:]
        in_maps.append({
            "xmT": np.ascontiguousarray(xm.T).astype(b),
            "xpmT": np.ascontiguousarray(xpm.T).astype(b),
            "xpppT": np.ascontiguousarray(xppp.T).astype(b),
            "xppmT": np.ascontiguousarray(xppm.T).astype(b),
            "wm": wm, "wpm": wpm, "wppp": wppp, "wppm": wppm,
        })
    return in_maps


def _bf16_to_f32(a: np.ndarray) -> np.ndarray:
    return (a.view(np.uint16).astype(np.uint32) << 16).view(np.float32)


def _recombine(dev_out: np.ndarray) -> np.ndarray:
    """[ROWS, D*1024] bf16 [Pm|Qm|Qpp|Qpm] partials -> f32 assoc."""
    arr = _bf16_to_f32(dev_out).reshape(ROWS, D, 1024)
    Pm = arr[:, :, :512]
    Qm = arr[:, :, 512:768]
    Qpp = arr[:, :, 768:896]
    Qpm = arr[:, :, 896:]
    y = np.empty((ROWS, D, 1024), np.float32)
    Qp_lo = y[:, :, 0:128]
    np.add(Qpp, Qpm, out=Qp_lo)
    Qp_hi = y[:, :, 128:256]
    np.subtract(Qpp, Qpm, out=Qp_hi)
    Qp = y[:, :, 0:256]
    t_hi = y[:, :, 256:512]
    np.subtract(Qp, Qm, out=t_hi)
    np.add(Qp, Qm, out=Qp)          # t_lo in place
    t = y[:, :, 0:512]
    np.subtract(t, Pm, out=y[:, :, 512:1024])
    np.add(t, Pm, out=t)
    return y.reshape(ROWS, D * 1024)


def kernel(problemhrr: np.ndarray, lemmahrr: np.ndarray,
           decoders: np.ndarray) -> np.ndarray:
    from concourse.bass_utils import run_bass_kernel_spmd

    problemhrr = np.asarray(problemhrr, dtype=np.float32)
    lemmahrr = np.asarray(lemmahrr, dtype=np.float32)
    decoders = np.asarray(decoders, dtype=np.float32)

    nc = _get_program()
    in_maps = _build_in_maps(problemhrr, lemmahrr, decoders)
    res = run_bass_kernel_spmd(nc, in_maps, list(range(NCORES)))

    full = np.empty((B, OUT_COLS), np.float32)
    full[:, :HRR] = problemhrr
    full[:, HRR:2 * HRR] = lemmahrr
    for c in range(NCORES):
        y = _recombine(res.results[c]["out"])
        rows = slice(c * BPC, (c + 1) * BPC)
        full[rows, 2 * HRR:2 * HRR + DN] = y[:BPC]
        full[rows, 2 * HRR + DN:] = y[BPC:]
    return full


# revision 11
# speedup vs baseline: 1.1464x; 1.0934x over previous
"""Trainium2 kernel for nn_Decoder_featurizer: HRR decoder featurization.

reference: out = concat([p, l, assoc(dec_d, p)..., assoc(dec_d, l)...], -1)
where assoc(d, x)[j] = sum_t d[t] * x[(j+t) % N]  (circular correlation).

Circular correlation is a circulant matmul y = x @ C_d with
C_d[k, j] = dec[d, (k-j) % N].  v5 cuts the PE work 3.5x below the dense
circulant by composing two matrix identities, recursively:

 (1) CRT split of a cyclic ring:  x^2M-1 = (x^M-1)(x^M+1) turns a
     circulant-2M matmul into a circulant-M plus a negacyclic-M matmul on
     folded inputs (2 half-size mults instead of 4 quarters).
 (2) Karatsuba on the negacyclic ring: the skew-circulant has block form
     [[P, -R], [R, P]] (complex-multiplication structure), so it costs 3
     half-size Toeplitz matmuls (m1=x0@P, m2=x1@R, m3=(x0+x1)@(P-R)) with
     the butterfly  y = [m1+m2, m3-m1+m2].

Applied to N=1024:  cyc1024 -> cyc512 + neg512;  cyc512 -> cyc256 + neg256;
cyc256 -> cyc128 + neg128;  neg512 -> 3 x Toep256 (Karatsuba);
neg256 -> 3 x Toep128 (Karatsuba).  Leaves per decoder:
  m1,m2,m3   : 3 matmuls [B,256]@[256,256]   (neg512 Karatsuba)
  n1,n2,n3   : 3 matmuls [B,128]@[128,128]   (neg256 Karatsuba)
  q          : [B,128]@circ128               (cyc128)
  s          : [B,128]@skew128               (neg128)
= 229376 MACs/row/decoder vs 1048576 dense.  All folded x operands are
host-precomputed (shipped transposed, bf16); all butterfly recombines run
on the host in f32.  The passthrough columns never touch the device.

Every device matmul has N=512: weights are stored decoder-contiguous so one
moving operand covers 2 decoders (256-wide leaves) or 4 decoders (128-wide
leaves) -- small-N matmuls measurably pay a large per-MM floor (v4 post-
mortem).  Per (m-tile, group-of-4-decoders): 3 PSUM phases (4+4+3 banks)
rotating through 2 pool buffers; DVE and ScalarE split the psum->SBUF bf16
drains so each phase's banks free up within the next phase's PE time.
Device output is raw partials in bf16 (group-major blocks, 1.4 MB DMA per
group).  Redundant LDWEIGHTS (consecutive MMs share a stationary x-tile)
are deduped at the BIR level.
"""

import numpy as np
import ml_dtypes

HRR = 1024
D = 16
B = 8192
NCORES = 8
BPC = B // NCORES            # batch rows per core
ROWS = 2 * BPC               # rows per core (problem + lemma stacked)
DN = D * HRR                 # 16384 assoc features per input
OUT_COLS = 2 * HRR + 2 * DN  # 34816
GRP_COLS = 5632              # device partial columns per 4-decoder group
DEV_COLS = 4 * GRP_COLS     # 22528 device partial columns per row

_CACHE = {}


def _build_program(loop_iters: int = 1):
    import contextlib
    import concourse.bacc as bacc
    import concourse.mybir as mybir
    from concourse.tile import TileContext

    nc = bacc.Bacc("TRN2", target_bir_lowering=False, debug=False,
                   num_devices=NCORES)
    bf16 = mybir.dt.bfloat16
    xin = {}
    for name in ("xm0", "xm1", "xms"):
        xin[name] = nc.dram_tensor(name, [256, ROWS], bf16,
                                   kind="ExternalInput").ap()
    for name in ("xpm0", "xpm1", "xpms", "xppp", "xppm"):
        xin[name] = nc.dram_tensor(name, [128, ROWS], bf16,
                                   kind="ExternalInput").ap()
    wk1 = nc.dram_tensor("wk1", [128, 2 * 3 * D * 256], bf16,
                         kind="ExternalInput").ap()
    wk2 = nc.dram_tensor("wk2", [128, 3 * D * 128], bf16,
                         kind="ExternalInput").ap()
    wk3 = nc.dram_tensor("wk3", [128, 2 * D * 128], bf16,
                         kind="ExternalInput").ap()
    out = nc.dram_tensor("out", [ROWS, DEV_COLS], bf16,
                         kind="ExternalOutput").ap()

    with TileContext(nc) as tc:
        with (
            tc.tile_pool(name="xp", bufs=1) as xpool,
            tc.tile_pool(name="wp", bufs=1) as wpool,
            tc.tile_pool(name="ps", bufs=2, space="PSUM") as pspool,
            tc.tile_pool(name="ob", bufs=3) as opool,
        ):
            # resident transposed folded activations
            xt = {}
            for name in ("xm0", "xm1", "xms"):
                tiles = []
                for k in range(2):
                    t = xpool.tile([128, ROWS], bf16, tag=f"{name}_{k}")
                    nc.sync.dma_start(
                        out=t[:], in_=xin[name][k * 128:(k + 1) * 128, :])
                    tiles.append(t)
                xt[name] = tiles
            for name in ("xpm0", "xpm1", "xpms", "xppp", "xppm"):
                t = xpool.tile([128, ROWS], bf16, tag=name)
                nc.sync.dma_start(out=t[:], in_=xin[name][:, :])
                xt[name] = t

            # resident weights (decoder-contiguous direct row blocks)
            wk1t = wpool.tile([128, 2 * 3 * D * 256], bf16, tag="wk1")
            nc.sync.dma_start(out=wk1t[:], in_=wk1[:, :])
            wk2t = wpool.tile([128, 3 * D * 128], bf16, tag="wk2")
            nc.sync.dma_start(out=wk2t[:], in_=wk2[:, :])
            wk3t = wpool.tile([128, 2 * D * 128], bf16, tag="wk3")
            nc.sync.dma_start(out=wk3t[:], in_=wk3[:, :])

            loop_cm = (tc.For_i(0, loop_iters, 1,
                                hint_engines=(mybir.EngineType.PE,
                                              mybir.EngineType.SP,
                                              mybir.EngineType.DVE,
                                              mybir.EngineType.Activation))
                       if loop_iters > 1 else contextlib.nullcontext())
            with loop_cm:
              for m in range(16):
                ms = slice(m * 128, (m + 1) * 128)
                for g in range(4):
                    d0 = 4 * g
                    stage = opool.tile([128, GRP_COLS], bf16, name="stage")

                    # phase 0: m1 (banks 0-1), m2 (banks 2-3)
                    ps0 = pspool.tile([128, 2048], mybir.dt.float32,
                                      name="ps")
                    for k in range(2):
                        lhsT = xt["xm0"][k][:, ms]
                        for p in range(2):
                            w0 = ((k * 3 + 0) * D + d0 + 2 * p) * 256
                            nc.tensor.matmul(
                                ps0[:, p * 512:(p + 1) * 512], lhsT,
                                wk1t[:, w0:w0 + 512],
                                start=(k == 0), stop=(k == 1))
                        lhsT = xt["xm1"][k][:, ms]
                        for p in range(2):
                            w0 = ((k * 3 + 1) * D + d0 + 2 * p) * 256
                            nc.tensor.matmul(
                                ps0[:, 1024 + p * 512:1536 + p * 512], lhsT,
                                wk1t[:, w0:w0 + 512],
                                start=(k == 0), stop=(k == 1))
                    nc.vector.tensor_copy(out=stage[:, 0:1024],
                                          in_=ps0[:, 0:1024])
                    nc.scalar.copy(out=stage[:, 1024:2048],
                                   in_=ps0[:, 1024:2048])

                    # phase 1: m3 (banks 0-1), n1 (bank 2), n2 (bank 3)
                    ps1 = pspool.tile([128, 2048], mybir.dt.float32,
                                      name="ps")
                    for k in range(2):
                        lhsT = xt["xms"][k][:, ms]
                        for p in range(2):
                            w0 = ((k * 3 + 2) * D + d0 + 2 * p) * 256
                            nc.tensor.matmul(
                                ps1[:, p * 512:(p + 1) * 512], lhsT,
                                wk1t[:, w0:w0 + 512],
                                start=(k == 0), stop=(k == 1))
                    nc.tensor.matmul(
                        ps1[:, 1024:1536], xt["xpm0"][:, ms],
                        wk2t[:, (0 * D + d0) * 128:(0 * D + d0) * 128 + 512],
                        start=True, stop=True)
                    nc.tensor.matmul(
                        ps1[:, 1536:2048], xt["xpm1"][:, ms],
                        wk2t[:, (1 * D + d0) * 128:(1 * D + d0) * 128 + 512],
                        start=True, stop=True)
                    nc.vector.tensor_copy(out=stage[:, 2048:3072],
                                          in_=ps1[:, 0:1024])
                    nc.scalar.copy(out=stage[:, 3072:4096],
                                   in_=ps1[:, 1024:2048])

                    # phase 2: n3 (bank 0), q (bank 1), s (bank 2)
                    ps2 = pspool.tile([128, 2048], mybir.dt.float32,
                                      name="ps")
                    nc.tensor.matmul(
                        ps2[:, 0:512], xt["xpms"][:, ms],
                        wk2t[:, (2 * D + d0) * 128:(2 * D + d0) * 128 + 512],
                        start=True, stop=True)
                    nc.tensor.matmul(
                        ps2[:, 512:1024], xt["xppp"][:, ms],
                        wk3t[:, (0 * D + d0) * 128:(0 * D + d0) * 128 + 512],
                        start=True, stop=True)
                    nc.tensor.matmul(
                        ps2[:, 1024:1536], xt["xppm"][:, ms],
                        wk3t[:, (1 * D + d0) * 128:(1 * D + d0) * 128 + 512],
                        start=True, stop=True)
                    nc.scalar.copy(out=stage[:, 4096:5632],
                                   in_=ps2[:, 0:1536])

                    nc.sync.dma_start(
                        out=out[ms, g * GRP_COLS:(g + 1) * GRP_COLS],
                        in_=stage[:])
    _finalize_with_dedup(nc)
    return nc


def _dedup_ldweights(nc):
    """Drop redundant InstLdweights from the PE stream.

    bacc emits every matmul as an (InstLdweights, InstMatmult) pair; the
    matmult is non-self-loading, so the PE weight register persists across
    matmuls.  Consecutive pairs with an identical stationary AP reload the
    same weights (~107ns each on HW).  Drop an InstLdweights when its
    signature matches the previous one on the PE stream AND it carries no
    waits/updates.  Conservatively resets tracking at block boundaries and
    on any other PE instruction.
    """
    import concourse.mybir as mybir

    InstLdweights = mybir.InstLdweights
    InstMatmult = mybir.InstMatmult
    n_drop = 0
    for fn in nc.m.functions:
        for blk in fn.blocks:
            keep = []
            last_sig = None
            for inst in blk.instructions:
                if isinstance(inst, InstLdweights):
                    pap = inst.ins[0]
                    sig = (pap.memref, pap.offset, str(pap.ap),
                           str(pap.dtype),
                           str(getattr(inst, "perf_mode", None)),
                           str(getattr(inst, "is_transpose", None)),
                           str(getattr(inst, "tile_position", None)))
                    si = inst.sync_info
                    bare = si is None or (len(si.on_wait) == 0
                                          and len(si.on_update) == 0)
                    if sig == last_sig and bare:
                        n_drop += 1
                        continue
                    last_sig = sig
                elif getattr(inst, "engine", None) == mybir.EngineType.PE:
                    if isinstance(inst, InstMatmult):
                        if getattr(inst, "is_transpose", None):
                            last_sig = None
                    else:
                        last_sig = None
                keep.append(inst)
            if n_drop:
                try:
                    blk.instructions = keep
                except Exception:
                    insts = blk.instructions
                    while len(insts):
                        insts.pop()
                    for i in keep:
                        insts.append(i)
    return n_drop


def _finalize_with_dedup(nc):
    orig_mv = nc.move_matmul_waits_to_ldweights

    def _mv():
        orig_mv()
        _dedup_ldweights(nc)

    nc.move_matmul_waits_to_ldweights = _mv
    nc.finalize()


def _get_program(loop_iters: int = 1):
    key = f"nc{loop_iters}"
    if key not in _CACHE:
        _CACHE[key] = _build_program(loop_iters)
    return _CACHE[key]


def _skew(v):
    """Skew-circulant (negacyclic) matrix W[k,j] = v[k-j], -v[k-j+M] below
    the diagonal."""
    M = len(v)
    k = np.arange(M)[:, None]
    j = np.arange(M)[None, :]
    r = k - j
    return np.where(r >= 0, v[r % M], -v[r % M])


def _circ(v):
    M = len(v)
    k = np.arange(M)[:, None]
    j = np.arange(M)[None, :]
    return v[(k - j) % M]


def _build_weights(decoders: np.ndarray):
    """Decoder-contiguous direct row-block weight buffers (bf16).

    wk1[:, ((k*3+mat)*D + d)*256 :][:256]: k-th 128-row block of the 256x256
    Karatsuba matrix mat in {P, R, P-R} of skew512(h_d)/2.
    wk2[:, (mat*D + d)*128 :][:128]: 128x128 Karatsuba matrix of
    skew256(gm_d)/4.   wk3: {circ128(gpp_d)/8, skew128(gpm_d)/8}.
    """
    dec = np.asarray(decoders, np.float32)
    wk1 = np.empty((128, 2 * 3 * D * 256), np.float32)
    wk2 = np.empty((128, 3 * D * 128), np.float32)
    wk3 = np.empty((128, 2 * D * 128), np.float32)
    for d in range(D):
        v = dec[d]
        g = v[:512] + v[512:]
        h = (v[:512] - v[512:]) / 2
        gm = (g[:256] - g[256:]) / 4
        gp = g[:256] + g[256:]
        gpp = (gp[:128] + gp[128:]) / 8
        gpm = (gp[:128] - gp[128:]) / 8

        S = _skew(h)                      # 512x512
        mats1 = (S[:256, :256], S[256:, :256],
                 S[:256, :256] - S[256:, :256])     # P, R, P-R
        for mat, W in enumerate(mats1):
            for k in range(2):
                c0 = ((k * 3 + mat) * D + d) * 256
                wk1[:, c0:c0 + 256] = W[k * 128:(k + 1) * 128, :]
        S2 = _skew(gm)                    # 256x256
        mats2 = (S2[:128, :128], S2[128:, :128],
                 S2[:128, :128] - S2[128:, :128])
        for mat, W in enumerate(mats2):
            c0 = (mat * D + d) * 128
            wk2[:, c0:c0 + 128] = W
        for which, W in enumerate((_circ(gpp), _skew(gpm))):
            c0 = (which * D + d) * 128
            wk3[:, c0:c0 + 128] = W
    b = ml_dtypes.bfloat16
    return wk1.astype(b), wk2.astype(b), wk3.astype(b)


def _build_in_maps(problemhrr, lemmahrr, decoders):
    b = ml_dtypes.bfloat16
    wk1, wk2, wk3 = _build_weights(decoders)

    def t(a):
        return np.ascontiguousarray(a.T).astype(b)

    in_maps = []
    for c in range(NCORES):
        p = problemhrr[c * BPC:(c + 1) * BPC]
        l = lemmahrr[c * BPC:(c + 1) * BPC]
        X = np.concatenate([p, l], axis=0)          # [2048, 1024] f32
        xm = X[:, :512] - X[:, 512:]
        xp = X[:, :512] + X[:, 512:]
        xpp = xp[:, :256] + xp[:, 256:]
        xpm = xp[:, :256] - xp[:, 256:]
        xm0, xm1 = xm[:, :256], xm[:, 256:]
        xpm0, xpm1 = xpm[:, :128], xpm[:, 128:]
        in_maps.append({
            "xm0": t(xm0), "xm1": t(xm1), "xms": t(xm0 + xm1),
            "xpm0": t(xpm0), "xpm1": t(xpm1), "xpms": t(xpm0 + xpm1),
            "xppp": t(xpp[:, :128] + xpp[:, 128:]),
            "xppm": t(xpp[:, :128] - xpp[:, 128:]),
            "wk1": wk1, "wk2": wk2, "wk3": wk3,
        })
    return in_maps


def _bf16_to_f32(a: np.ndarray) -> np.ndarray:
    return (a.view(np.uint16).astype(np.uint32) << 16).view(np.float32)


def _recombine(dev_out: np.ndarray) -> np.ndarray:
    """[ROWS, DEV_COLS] bf16 group-major partials -> [ROWS, D*1024] f32."""
    blk = _bf16_to_f32(dev_out).reshape(ROWS, 4, GRP_COLS)
    m1 = blk[:, :, 0:1024].reshape(ROWS, 4, 4, 256)
    m2 = blk[:, :, 1024:2048].reshape(ROWS, 4, 4, 256)
    m3 = blk[:, :, 2048:3072].reshape(ROWS, 4, 4, 256)
    n1 = blk[:, :, 3072:3584].reshape(ROWS, 4, 4, 128)
    n2 = blk[:, :, 3584:4096].reshape(ROWS, 4, 4, 128)
    n3 = blk[:, :, 4096:4608].reshape(ROWS, 4, 4, 128)
    q = blk[:, :, 4608:5120].reshape(ROWS, 4, 4, 128)
    s = blk[:, :, 5120:5632].reshape(ROWS, 4, 4, 128)

    y = np.empty((ROWS, 4, 4, 1024), np.float32)
    t = y[:, :, :, 0:512]
    Qp_lo = y[:, :, :, 0:128]           # scratch inside t
    np.add(q, s, out=Qp_lo)
    Qp_hi = y[:, :, :, 128:256]
    np.subtract(q, s, out=Qp_hi)
    Qp = y[:, :, :, 0:256]
    Qm = np.concatenate([n1 + n2, n3 - n1 + n2], axis=-1)   # [.,.,.,256]
    np.subtract(Qp, Qm, out=y[:, :, :, 256:512])
    np.add(Qp, Qm, out=Qp)              # t = [Qp+Qm, Qp-Qm]
    Pm = np.concatenate([m1 + m2, m3 - m1 + m2], axis=-1)   # [.,.,.,512]
    np.subtract(t, Pm, out=y[:, :, :, 512:1024])
    np.add(t, Pm, out=t)
    return y.reshape(ROWS, D * 1024)


def kernel(problemhrr: np.ndarray, lemmahrr: np.ndarray,
           decoders: np.ndarray) -> np.ndarray:
    from concourse.bass_utils import run_bass_kernel_spmd

    problemhrr = np.asarray(problemhrr, dtype=np.float32)
    lemmahrr = np.asarray(lemmahrr, dtype=np.float32)
    decoders = np.asarray(decoders, dtype=np.float32)

    nc = _get_program()
    in_maps = _build_in_maps(problemhrr, lemmahrr, decoders)
    res = run_bass_kernel_spmd(nc, in_maps, list(range(NCORES)))

    full = np.empty((B, OUT_COLS), np.float32)
    full[:, :HRR] = problemhrr
    full[:, HRR:2 * HRR] = lemmahrr
    for c in range(NCORES):
        y = _recombine(res.results[c]["out"])
        rows = slice(c * BPC, (c + 1) * BPC)
        full[rows, 2 * HRR:2 * HRR + DN] = y[:BPC]
        full[rows, 2 * HRR + DN:] = y[BPC:]
    return full


# revision 20
# speedup vs baseline: 1.2056x; 1.0516x over previous
"""Trainium2 kernel for nn_Decoder_featurizer: HRR decoder featurization.

reference: out = concat([p, l, assoc(dec_d, p)..., assoc(dec_d, l)...], -1)
where assoc(d, x)[j] = sum_t d[t] * x[(j+t) % N]  (circular correlation).

Circular correlation is a circulant matmul y = x @ C_d with
C_d[k, j] = dec[d, (k-j) % N].  v5 cuts the PE work 3.5x below the dense
circulant by composing two matrix identities, recursively:

 (1) CRT split of a cyclic ring:  x^2M-1 = (x^M-1)(x^M+1) turns a
     circulant-2M matmul into a circulant-M plus a negacyclic-M matmul on
     folded inputs (2 half-size mults instead of 4 quarters).
 (2) Karatsuba on the negacyclic ring: the skew-circulant has block form
     [[P, -R], [R, P]] (complex-multiplication structure), so it costs 3
     half-size Toeplitz matmuls (m1=x0@P, m2=x1@R, m3=(x0+x1)@(P-R)) with
     the butterfly  y = [m1+m2, m3-m1+m2].

Applied to N=1024:  cyc1024 -> cyc512 + neg512;  cyc512 -> cyc256 + neg256;
cyc256 -> cyc128 + neg128;  neg512 -> 3 x Toep256 (Karatsuba);
neg256 -> 3 x Toep128 (Karatsuba).  Leaves per decoder:
  m1,m2,m3   : 3 matmuls [B,256]@[256,256]   (neg512 Karatsuba)
  n1,n2,n3   : 3 matmuls [B,128]@[128,128]   (neg256 Karatsuba)
  q          : [B,128]@circ128               (cyc128)
  s          : [B,128]@skew128               (neg128)
= 229376 MACs/row/decoder vs 1048576 dense.  All folded x operands are
host-precomputed (shipped transposed, bf16); all butterfly recombines run
on the host in f32.  The passthrough columns never touch the device.

Every device matmul has N=512: weights are stored decoder-contiguous so one
moving operand covers 2 decoders (256-wide leaves) or 4 decoders (128-wide
leaves) -- small-N matmuls measurably pay a large per-MM floor (v4 post-
mortem).  v7 organizes the work as single-leaf PSUM phases spanning 8
decoders (256-wide leaves, 4 pairs) or all 16 (128-wide leaves), so each
stationary LDWEIGHTS feeds 4 N=512 matmuls (LDWEIGHTS is serial PE time at
~107ns; it was 24%% of the PE budget at 1-2 MMs per load).  Per m-tile: 11
phases of <=4 PSUM banks rotating through 2 pool buffers; DVE and ScalarE
split each phase's psum->SBUF bf16 drain so banks free within the next
phase's PE time; each phase block DMAs out directly (0.5-1 MB).  Device
output is raw partials in bf16, phase-major.  Redundant LDWEIGHTS are
deduped at the BIR level.
"""

import numpy as np
import ml_dtypes

HRR = 1024
D = 16
B = 8192
NCORES = 8
BPC = B // NCORES            # batch rows per core
ROWS = 2 * BPC               # rows per core (problem + lemma stacked)
DN = D * HRR                 # 16384 assoc features per input
OUT_COLS = 2 * HRR + 2 * DN  # 34816
DEV_COLS = 22528             # device partial columns per row
# phase-major column layout: m1,m2,m3 [16 dec x 256], n1,n2,n3,q,s [16 x 128]
_PHASE_COLS = (("m1", 4096), ("m2", 4096), ("m3", 4096), ("n1", 2048),
               ("n2", 2048), ("n3", 2048), ("q", 2048), ("s", 2048))

_CACHE = {}


def _build_program(loop_iters: int = 1, pe_only: bool = False):
    """pe_only=True builds a timing-diagnostic variant with the drain
    copies and output DMAs removed from the loop (output stays zero)."""
    import contextlib
    import concourse.bacc as bacc
    import concourse.mybir as mybir
    from concourse.tile import TileContext

    nc = bacc.Bacc("TRN2", target_bir_lowering=False, debug=False,
                   num_devices=NCORES)
    bf16 = mybir.dt.bfloat16
    xin = {}
    for name in ("xm0", "xm1", "xms"):
        xin[name] = nc.dram_tensor(name, [256, ROWS], bf16,
                                   kind="ExternalInput").ap()
    for name in ("xpm0", "xpm1", "xpms", "xppp", "xppm"):
        xin[name] = nc.dram_tensor(name, [128, ROWS], bf16,
                                   kind="ExternalInput").ap()
    wk1 = nc.dram_tensor("wk1", [128, 2 * 3 * D * 256], bf16,
                         kind="ExternalInput").ap()
    wk2 = nc.dram_tensor("wk2", [128, 3 * D * 128], bf16,
                         kind="ExternalInput").ap()
    wk3 = nc.dram_tensor("wk3", [128, 2 * D * 128], bf16,
                         kind="ExternalInput").ap()
    out = nc.dram_tensor("out", [ROWS, DEV_COLS], bf16,
                         kind="ExternalOutput").ap()

    with TileContext(nc) as tc:
        with (
            tc.tile_pool(name="xp", bufs=1) as xpool,
            tc.tile_pool(name="wp", bufs=1) as wpool,
            tc.tile_pool(name="ps", bufs=2, space="PSUM") as pspool,
            tc.tile_pool(name="ob", bufs=3) as opool,
        ):
            # resident transposed folded activations
            xt = {}
            for name in ("xm0", "xm1", "xms"):
                tiles = []
                for k in range(2):
                    t = xpool.tile([128, ROWS], bf16, tag=f"{name}_{k}")
                    nc.sync.dma_start(
                        out=t[:], in_=xin[name][k * 128:(k + 1) * 128, :])
                    tiles.append(t)
                xt[name] = tiles
            for name in ("xpm0", "xpm1", "xpms", "xppp", "xppm"):
                t = xpool.tile([128, ROWS], bf16, tag=name)
                nc.sync.dma_start(out=t[:], in_=xin[name][:, :])
                xt[name] = t

            # resident weights (decoder-contiguous direct row blocks)
            wk1t = wpool.tile([128, 2 * 3 * D * 256], bf16, tag="wk1")
            nc.sync.dma_start(out=wk1t[:], in_=wk1[:, :])
            wk2t = wpool.tile([128, 3 * D * 128], bf16, tag="wk2")
            nc.sync.dma_start(out=wk2t[:], in_=wk2[:, :])
            wk3t = wpool.tile([128, 2 * D * 128], bf16, tag="wk3")
            nc.sync.dma_start(out=wk3t[:], in_=wk3[:, :])

            loop_cm = (tc.For_i(0, loop_iters, 1,
                                hint_engines=(mybir.EngineType.PE,
                                              mybir.EngineType.SP,
                                              mybir.EngineType.DVE,
                                              mybir.EngineType.Activation))
                       if loop_iters > 1 else contextlib.nullcontext())
            with loop_cm:
              for m in range(16):
                ms = slice(m * 128, (m + 1) * 128)
                # sequential 2048-col phase drains accumulate into a shared
                # [128, 4096] stage tile; flush 1 MB DMAs when full
                st = {"tile": None, "fill": 0, "col": 0, "flip": 0}

                def drain_dma(ps, width):
                    """Split the phase drain DVE/ACT so banks free early;
                    DMA out whenever the stage tile fills."""
                    if pe_only:
                        return
                    if st["tile"] is None:
                        st["tile"] = opool.tile([128, 4096], bf16,
                                                name="stage")
                        st["fill"] = 0
                    stage, f0 = st["tile"], st["fill"]
                    half = width // 2
                    engs = (nc.vector.tensor_copy, nc.scalar.copy)
                    f = st["flip"]
                    st["flip"] ^= 1
                    engs[f](out=stage[:, f0:f0 + half], in_=ps[:, 0:half])
                    engs[f ^ 1](out=stage[:, f0 + half:f0 + width],
                                in_=ps[:, half:width])
                    st["fill"] += width
                    if st["fill"] >= 4096:
                        nc.sync.dma_start(
                            out=out[ms, st["col"]:st["col"] + st["fill"]],
                            in_=stage[:, 0:st["fill"]])
                        st["col"] += st["fill"]
                        st["tile"] = None

                # m-leaves (K=256): one phase per Karatsuba matrix per
                # 8-decoder half; each LDWEIGHTS feeds 4 N=512 matmuls.
                xmk = (xt["xm0"], xt["xm1"], xt["xms"])
                for mat in range(3):
                    for h in range(2):
                        ps = pspool.tile([128, 2048], mybir.dt.float32,
                                         name="ps")
                        for k in range(2):
                            lhsT = xmk[mat][k][:, ms]
                            for p in range(4):
                                w0 = ((k * 3 + mat) * D + 8 * h
                                      + 2 * p) * 256
                                nc.tensor.matmul(
                                    ps[:, p * 512:(p + 1) * 512], lhsT,
                                    wk1t[:, w0:w0 + 512],
                                    start=(k == 0), stop=(k == 1))
                        drain_dma(ps, 2048)

                # 128-wide leaves (K=128): one phase per leaf spanning all
                # 16 decoders; one LDWEIGHTS feeds 4 N=512 matmuls.
                leaves = (
                    (xt["xpm0"], wk2t, 0), (xt["xpm1"], wk2t, 1),
                    (xt["xpms"], wk2t, 2), (xt["xppp"], wk3t, 0),
                    (xt["xppm"], wk3t, 1),
                )
                for lx, wt, mat in leaves:
                    ps = pspool.tile([128, 2048], mybir.dt.float32,
                                     name="ps")
                    lhsT = lx[:, ms]
                    for p in range(4):
                        w0 = (mat * D + 4 * p) * 128
                        nc.tensor.matmul(
                            ps[:, p * 512:(p + 1) * 512], lhsT,
                            wt[:, w0:w0 + 512], start=True, stop=True)
                    drain_dma(ps, 2048)
                # flush the trailing half-filled stage (the s block)
                if st["tile"] is not None:
                    nc.sync.dma_start(
                        out=out[ms, st["col"]:st["col"] + st["fill"]],
                        in_=st["tile"][:, 0:st["fill"]])
                    st["tile"] = None
    _finalize_with_dedup(nc)
    return nc


def _dedup_ldweights(nc):
    """Drop redundant InstLdweights from the PE stream.

    bacc emits every matmul as an (InstLdweights, InstMatmult) pair; the
    matmult is non-self-loading, so the PE weight register persists across
    matmuls.  Consecutive pairs with an identical stationary AP reload the
    same weights (~107ns each on HW).  Drop an InstLdweights when its
    signature matches the previous one on the PE stream AND it carries no
    waits/updates.  Conservatively resets tracking at block boundaries and
    on any other PE instruction.
    """
    import concourse.mybir as mybir

    InstLdweights = mybir.InstLdweights
    InstMatmult = mybir.InstMatmult
    n_drop = 0
    for fn in nc.m.functions:
        for blk in fn.blocks:
            keep = []
            last_sig = None
            for inst in blk.instructions:
                if isinstance(inst, InstLdweights):
                    pap = inst.ins[0]
                    sig = (pap.memref, pap.offset, str(pap.ap),
                           str(pap.dtype),
                           str(getattr(inst, "perf_mode", None)),
                           str(getattr(inst, "is_transpose", None)),
                           str(getattr(inst, "tile_position", None)))
                    si = inst.sync_info
                    bare = si is None or (len(si.on_wait) == 0
                                          and len(si.on_update) == 0)
                    if sig == last_sig and bare:
                        n_drop += 1
                        continue
                    last_sig = sig
                elif getattr(inst, "engine", None) == mybir.EngineType.PE:
                    if isinstance(inst, InstMatmult):
                        if getattr(inst, "is_transpose", None):
                            last_sig = None
                    else:
                        last_sig = None
                keep.append(inst)
            if n_drop:
                try:
                    blk.instructions = keep
                except Exception:
                    insts = blk.instructions
                    while len(insts):
                        insts.pop()
                    for i in keep:
                        insts.append(i)
    return n_drop


def _finalize_with_dedup(nc):
    orig_mv = nc.move_matmul_waits_to_ldweights

    def _mv():
        orig_mv()
        _dedup_ldweights(nc)

    nc.move_matmul_waits_to_ldweights = _mv
    nc.finalize()


def _get_program(loop_iters: int = 1):
    key = f"nc{loop_iters}"
    if key not in _CACHE:
        _CACHE[key] = _build_program(loop_iters)
    return _CACHE[key]


def _skew(v):
    """Skew-circulant (negacyclic) matrix W[k,j] = v[k-j], -v[k-j+M] below
    the diagonal."""
    M = len(v)
    k = np.arange(M)[:, None]
    j = np.arange(M)[None, :]
    r = k - j
    return np.where(r >= 0, v[r % M], -v[r % M])


def _circ(v):
    M = len(v)
    k = np.arange(M)[:, None]
    j = np.arange(M)[None, :]
    return v[(k - j) % M]


def _build_weights(decoders: np.ndarray):
    """Decoder-contiguous direct row-block weight buffers (bf16).

    wk1[:, ((k*3+mat)*D + d)*256 :][:256]: k-th 128-row block of the 256x256
    Karatsuba matrix mat in {P, R, P-R} of skew512(h_d)/2.
    wk2[:, (mat*D + d)*128 :][:128]: 128x128 Karatsuba matrix of
    skew256(gm_d)/4.   wk3: {circ128(gpp_d)/8, skew128(gpm_d)/8}.
    """
    dec = np.asarray(decoders, np.float32)
    wk1 = np.empty((128, 2 * 3 * D * 256), np.float32)
    wk2 = np.empty((128, 3 * D * 128), np.float32)
    wk3 = np.empty((128, 2 * D * 128), np.float32)
    for d in range(D):
        v = dec[d]
        g = v[:512] + v[512:]
        h = (v[:512] - v[512:]) / 2
        gm = (g[:256] - g[256:]) / 4
        gp = g[:256] + g[256:]
        gpp = (gp[:128] + gp[128:]) / 8
        gpm = (gp[:128] - gp[128:]) / 8

        S = _skew(h)                      # 512x512
        mats1 = (S[:256, :256], S[256:, :256],
                 S[:256, :256] - S[256:, :256])     # P, R, P-R
        for mat, W in enumerate(mats1):
            for k in range(2):
                c0 = ((k * 3 + mat) * D + d) * 256
                wk1[:, c0:c0 + 256] = W[k * 128:(k + 1) * 128, :]
        S2 = _skew(gm)                    # 256x256
        mats2 = (S2[:128, :128], S2[128:, :128],
                 S2[:128, :128] - S2[128:, :128])
        for mat, W in enumerate(mats2):
            c0 = (mat * D + d) * 128
            wk2[:, c0:c0 + 128] = W
        for which, W in enumerate((_circ(gpp), _skew(gpm))):
            c0 = (which * D + d) * 128
            wk3[:, c0:c0 + 128] = W
    b = ml_dtypes.bfloat16
    return wk1.astype(b), wk2.astype(b), wk3.astype(b)


def _build_in_maps(problemhrr, lemmahrr, decoders):
    b = ml_dtypes.bfloat16
    wk1, wk2, wk3 = _build_weights(decoders)

    def t(a):
        return np.ascontiguousarray(a.T).astype(b)

    in_maps = []
    for c in range(NCORES):
        p = problemhrr[c * BPC:(c + 1) * BPC]
        l = lemmahrr[c * BPC:(c + 1) * BPC]
        X = np.concatenate([p, l], axis=0)          # [2048, 1024] f32
        xm = X[:, :512] - X[:, 512:]
        xp = X[:, :512] + X[:, 512:]
        xpp = xp[:, :256] + xp[:, 256:]
        xpm = xp[:, :256] - xp[:, 256:]
        xm0, xm1 = xm[:, :256], xm[:, 256:]
        xpm0, xpm1 = xpm[:, :128], xpm[:, 128:]
        in_maps.append({
            "xm0": t(xm0), "xm1": t(xm1), "xms": t(xm0 + xm1),
            "xpm0": t(xpm0), "xpm1": t(xpm1), "xpms": t(xpm0 + xpm1),
            "xppp": t(xpp[:, :128] + xpp[:, 128:]),
            "xppm": t(xpp[:, :128] - xpp[:, 128:]),
            "wk1": wk1, "wk2": wk2, "wk3": wk3,
        })
    return in_maps


def _bf16_to_f32(a: np.ndarray) -> np.ndarray:
    return (a.view(np.uint16).astype(np.uint32) << 16).view(np.float32)


def _recombine(dev_out: np.ndarray) -> np.ndarray:
    """[ROWS, DEV_COLS] bf16 phase-major partials -> [ROWS, D*1024] f32."""
    arr = _bf16_to_f32(dev_out)
    m1 = arr[:, 0:4096].reshape(ROWS, D, 256)
    m2 = arr[:, 4096:8192].reshape(ROWS, D, 256)
    m3 = arr[:, 8192:12288].reshape(ROWS, D, 256)
    n1 = arr[:, 12288:14336].reshape(ROWS, D, 128)
    n2 = arr[:, 14336:16384].reshape(ROWS, D, 128)
    n3 = arr[:, 16384:18432].reshape(ROWS, D, 128)
    q = arr[:, 18432:20480].reshape(ROWS, D, 128)
    s = arr[:, 20480:22528].reshape(ROWS, D, 128)

    y = np.empty((ROWS, D, 1024), np.float32)
    t = y[:, :, 0:512]
    Qp_lo = y[:, :, 0:128]              # scratch inside t
    np.add(q, s, out=Qp_lo)
    Qp_hi = y[:, :, 128:256]
    np.subtract(q, s, out=Qp_hi)
    Qp = y[:, :, 0:256]
    Qm = np.concatenate([n1 + n2, n3 - n1 + n2], axis=-1)   # [.,.,256]
    np.subtract(Qp, Qm, out=y[:, :, 256:512])
    np.add(Qp, Qm, out=Qp)              # t = [Qp+Qm, Qp-Qm]
    Pm = np.concatenate([m1 + m2, m3 - m1 + m2], axis=-1)   # [.,.,512]
    np.subtract(t, Pm, out=y[:, :, 512:1024])
    np.add(t, Pm, out=t)
    return y.reshape(ROWS, D * 1024)


def kernel(problemhrr: np.ndarray, lemmahrr: np.ndarray,
           decoders: np.ndarray) -> np.ndarray:
    from concourse.bass_utils import run_bass_kernel_spmd

    problemhrr = np.asarray(problemhrr, dtype=np.float32)
    lemmahrr = np.asarray(lemmahrr, dtype=np.float32)
    decoders = np.asarray(decoders, dtype=np.float32)

    nc = _get_program()
    in_maps = _build_in_maps(problemhrr, lemmahrr, decoders)
    res = run_bass_kernel_spmd(nc, in_maps, list(range(NCORES)))

    full = np.empty((B, OUT_COLS), np.float32)
    full[:, :HRR] = problemhrr
    full[:, HRR:2 * HRR] = lemmahrr
    for c in range(NCORES):
        y = _recombine(res.results[c]["out"])
        rows = slice(c * BPC, (c + 1) * BPC)
        full[rows, 2 * HRR:2 * HRR + DN] = y[:BPC]
        full[rows, 2 * HRR + DN:] = y[BPC:]
    return full


# revision 32
# speedup vs baseline: 1.2282x; 1.0187x over previous
"""Trainium2 kernel for nn_Decoder_featurizer: HRR decoder featurization.

reference: out = concat([p, l, assoc(dec_d, p)..., assoc(dec_d, l)...], -1)
where assoc(d, x)[j] = sum_t d[t] * x[(j+t) % N]  (circular correlation).

Circular correlation is a circulant matmul y = x @ C_d with
C_d[k, j] = dec[d, (k-j) % N].  v5 cuts the PE work 3.5x below the dense
circulant by composing two matrix identities, recursively:

 (1) CRT split of a cyclic ring:  x^2M-1 = (x^M-1)(x^M+1) turns a
     circulant-2M matmul into a circulant-M plus a negacyclic-M matmul on
     folded inputs (2 half-size mults instead of 4 quarters).
 (2) Karatsuba on the negacyclic ring: the skew-circulant has block form
     [[P, -R], [R, P]] (complex-multiplication structure), so it costs 3
     half-size Toeplitz matmuls (m1=x0@P, m2=x1@R, m3=(x0+x1)@(P-R)) with
     the butterfly  y = [m1+m2, m3-m1+m2].

Applied to N=1024:  cyc1024 -> cyc512 + neg512;  cyc512 -> cyc256 + neg256;
cyc256 -> cyc128 + neg128;  neg512 -> 3 x Toep256 (Karatsuba).  Leaves per
decoder:
  m1,m2,m3     : 3 matmuls [B,256]@[256,256]   (neg512 Karatsuba)
  Qm_lo,Qm_hi  : 4 matmuls [B,128]@[128,128]   (neg256 direct block form,
                 pairs PSUM-accumulated: Qm_lo = x0@P2 + x1@R2,
                 Qm_hi = x1@P2 - x0@R2 -- trades 1 extra small matmul for
                 shipping 256 instead of 384 partial cols; the drain/DMA
                 path, not the PE, is the binding budget)
  q            : [B,128]@circ128               (cyc128)
  s            : [B,128]@skew128               (neg128)
= 245760 MACs/row/decoder vs 1048576 dense.  All folded x operands are
host-precomputed (shipped transposed, bf16); the remaining butterfly
recombines run on the host in f32.  The passthrough columns never touch
the device.

Every device matmul has N=512: weights are stored decoder-contiguous so one
moving operand covers 2 decoders (256-wide leaves) or 4 decoders (128-wide
leaves) -- small-N matmuls measurably pay a large per-MM floor (v4 post-
mortem).  v7 organizes the work as single-leaf PSUM phases spanning 8
decoders (256-wide leaves, 4 pairs) or all 16 (128-wide leaves), so each
stationary LDWEIGHTS feeds 4 N=512 matmuls.  Per m-tile: 10 phases of 4
PSUM banks rotating through 2 pool buffers; DVE and ScalarE split each
phase's psum->SBUF bf16 drain so banks free within the next phase's PE
window; all drains land in one stage tile and a single 5.2 MB DMA per
m-tile writes out at ~97%% DMA efficiency.  Device output is raw partials
in bf16, phase-major.  Redundant LDWEIGHTS are deduped at the BIR level.
"""

import numpy as np
import ml_dtypes

HRR = 1024
D = 16
B = 8192
NCORES = 8
BPC = B // NCORES            # batch rows per core
ROWS = 2 * BPC               # rows per core (problem + lemma stacked)
DN = D * HRR                 # 16384 assoc features per input
OUT_COLS = 2 * HRR + 2 * DN  # 34816
DEV_COLS = 20480             # device partial columns per row
# phase-major column layout: m1,m2,m3 [16 dec x 256], Qm_lo,Qm_hi,q,s [16x128]

_CACHE = {}


def _build_program(loop_iters: int = 1, pe_only: bool = False):
    """pe_only=True builds a timing-diagnostic variant with the drain
    copies and output DMAs removed from the loop (output stays zero)."""
    import contextlib
    import concourse.bacc as bacc
    import concourse.mybir as mybir
    from concourse.tile import TileContext

    nc = bacc.Bacc("TRN2", target_bir_lowering=False, debug=False,
                   num_devices=NCORES)
    bf16 = mybir.dt.bfloat16
    xin = {}
    for name in ("xm0", "xm1", "xms"):
        xin[name] = nc.dram_tensor(name, [256, ROWS], bf16,
                                   kind="ExternalInput").ap()
    for name in ("xpm0", "xpm1", "xppp", "xppm"):
        xin[name] = nc.dram_tensor(name, [128, ROWS], bf16,
                                   kind="ExternalInput").ap()
    wk1 = nc.dram_tensor("wk1", [128, 2 * 3 * D * 256], bf16,
                         kind="ExternalInput").ap()
    wk2 = nc.dram_tensor("wk2", [128, 3 * D * 128], bf16,
                         kind="ExternalInput").ap()
    wk3 = nc.dram_tensor("wk3", [128, 2 * D * 128], bf16,
                         kind="ExternalInput").ap()
    out = nc.dram_tensor("out", [ROWS, DEV_COLS], bf16,
                         kind="ExternalOutput").ap()

    with TileContext(nc) as tc:
        with (
            tc.tile_pool(name="xp", bufs=1) as xpool,
            tc.tile_pool(name="wp", bufs=1) as wpool,
            tc.tile_pool(name="ps", bufs=2, space="PSUM") as pspool,
            tc.tile_pool(name="ob", bufs=2) as opool,
        ):
            # resident transposed folded activations
            xt = {}
            for name in ("xm0", "xm1", "xms"):
                tiles = []
                for k in range(2):
                    t = xpool.tile([128, ROWS], bf16, tag=f"{name}_{k}")
                    nc.sync.dma_start(
                        out=t[:], in_=xin[name][k * 128:(k + 1) * 128, :])
                    tiles.append(t)
                xt[name] = tiles
            for name in ("xpm0", "xpm1", "xppp", "xppm"):
                t = xpool.tile([128, ROWS], bf16, tag=name)
                nc.sync.dma_start(out=t[:], in_=xin[name][:, :])
                xt[name] = t

            # resident weights (decoder-contiguous direct row blocks)
            wk1t = wpool.tile([128, 2 * 3 * D * 256], bf16, tag="wk1")
            nc.sync.dma_start(out=wk1t[:], in_=wk1[:, :])
            wk2t = wpool.tile([128, 3 * D * 128], bf16, tag="wk2")
            nc.sync.dma_start(out=wk2t[:], in_=wk2[:, :])
            wk3t = wpool.tile([128, 2 * D * 128], bf16, tag="wk3")
            nc.sync.dma_start(out=wk3t[:], in_=wk3[:, :])

            loop_cm = (tc.For_i(0, loop_iters, 1,
                                hint_engines=(mybir.EngineType.PE,
                                              mybir.EngineType.SP,
                                              mybir.EngineType.DVE,
                                              mybir.EngineType.Activation))
                       if loop_iters > 1 else contextlib.nullcontext())
            with loop_cm:
              for m in range(16):
                ms = slice(m * 128, (m + 1) * 128)
                # all 10 phase drains land in one stage tile; a single
                # 5.2 MB DMA per m-tile runs at ~97% DMA efficiency
                stage = (None if pe_only else
                         opool.tile([128, DEV_COLS], bf16, name="stage"))
                st = {"fill": 0, "flip": 0}

                def drain_dma(ps, width):
                    """Split each phase drain DVE/ACT so banks free within
                    the next phase's PE window."""
                    f0 = st["fill"]
                    st["fill"] += width
                    if pe_only:
                        return
                    half = width // 2
                    engs = (nc.vector.tensor_copy, nc.scalar.copy)
                    f = st["flip"]
                    st["flip"] ^= 1
                    engs[f](out=stage[:, f0:f0 + half], in_=ps[:, 0:half])
                    engs[f ^ 1](out=stage[:, f0 + half:f0 + width],
                                in_=ps[:, half:width])

                # m-leaves (K=256): one phase per Karatsuba matrix per
                # 8-decoder half; each LDWEIGHTS feeds 4 N=512 matmuls.
                xmk = (xt["xm0"], xt["xm1"], xt["xms"])
                for mat in range(3):
                    for h in range(2):
                        ps = pspool.tile([128, 2048], mybir.dt.float32,
                                         name="ps")
                        for k in range(2):
                            lhsT = xmk[mat][k][:, ms]
                            for p in range(4):
                                w0 = ((k * 3 + mat) * D + 8 * h
                                      + 2 * p) * 256
                                nc.tensor.matmul(
                                    ps[:, p * 512:(p + 1) * 512], lhsT,
                                    wk1t[:, w0:w0 + 512],
                                    start=(k == 0), stop=(k == 1))
                        drain_dma(ps, 2048)

                # neg256 direct block form, PSUM-accumulated (K=128 each):
                #   Qm_lo = xpm0 @ P2 + xpm1 @ R2
                #   Qm_hi = xpm1 @ P2 - xpm0 @ R2
                # (wk2 stores [P2 | R2 | -R2] 16-decoder blocks)
                for ops in (((xt["xpm0"], 0), (xt["xpm1"], 1)),
                            ((xt["xpm1"], 0), (xt["xpm0"], 2))):
                    ps = pspool.tile([128, 2048], mybir.dt.float32,
                                     name="ps")
                    for step, (lx, mat) in enumerate(ops):
                        lhsT = lx[:, ms]
                        for p in range(4):
                            w0 = (mat * D + 4 * p) * 128
                            nc.tensor.matmul(
                                ps[:, p * 512:(p + 1) * 512], lhsT,
                                wk2t[:, w0:w0 + 512],
                                start=(step == 0), stop=(step == 1))
                    drain_dma(ps, 2048)

                # cyc128/neg128 leaves (K=128): one phase per leaf spanning
                # all 16 decoders; one LDWEIGHTS feeds 4 N=512 matmuls.
                for lx, mat in ((xt["xppp"], 0), (xt["xppm"], 1)):
                    ps = pspool.tile([128, 2048], mybir.dt.float32,
                                     name="ps")
                    lhsT = lx[:, ms]
                    for p in range(4):
                        w0 = (mat * D + 4 * p) * 128
                        nc.tensor.matmul(
                            ps[:, p * 512:(p + 1) * 512], lhsT,
                            wk3t[:, w0:w0 + 512], start=True, stop=True)
                    drain_dma(ps, 2048)
                if not pe_only:
                    nc.sync.dma_start(out=out[ms, :], in_=stage[:])
    _finalize_with_dedup(nc)
    return nc


def _dedup_ldweights(nc):
    """Drop redundant InstLdweights from the PE stream.

    bacc emits every matmul as an (InstLdweights, InstMatmult) pair; the
    matmult is non-self-loading, so the PE weight register persists across
    matmuls.  Consecutive pairs with an identical stationary AP reload the
    same weights (~107ns each on HW).  Drop an InstLdweights when its
    signature matches the previous one on the PE stream AND it carries no
    waits/updates.  Conservatively resets tracking at block boundaries and
    on any other PE instruction.
    """
    import concourse.mybir as mybir

    InstLdweights = mybir.InstLdweights
    InstMatmult = mybir.InstMatmult
    n_drop = 0
    for fn in nc.m.functions:
        for blk in fn.blocks:
            keep = []
            last_sig = None
            for inst in blk.instructions:
                if isinstance(inst, InstLdweights):
                    pap = inst.ins[0]
                    sig = (pap.memref, pap.offset, str(pap.ap),
                           str(pap.dtype),
                           str(getattr(inst, "perf_mode", None)),
                           str(getattr(inst, "is_transpose", None)),
                           str(getattr(inst, "tile_position", None)))
                    si = inst.sync_info
                    bare = si is None or (len(si.on_wait) == 0
                                          and len(si.on_update) == 0)
                    if sig == last_sig and bare:
                        n_drop += 1
                        continue
                    last_sig = sig
                elif getattr(inst, "engine", None) == mybir.EngineType.PE:
                    if isinstance(inst, InstMatmult):
                        if getattr(inst, "is_transpose", None):
                            last_sig = None
                    else:
                        last_sig = None
                keep.append(inst)
            if n_drop:
                try:
                    blk.instructions = keep
                except Exception:
                    insts = blk.instructions
                    while len(insts):
                        insts.pop()
                    for i in keep:
                        insts.append(i)
    return n_drop


def _finalize_with_dedup(nc):
    orig_mv = nc.move_matmul_waits_to_ldweights

    def _mv():
        orig_mv()
        _dedup_ldweights(nc)

    nc.move_matmul_waits_to_ldweights = _mv
    nc.finalize()


def _get_program(loop_iters: int = 1):
    key = f"nc{loop_iters}"
    if key not in _CACHE:
        _CACHE[key] = _build_program(loop_iters)
    return _CACHE[key]


def _skew(v):
    """Skew-circulant (negacyclic) matrix W[k,j] = v[k-j], -v[k-j+M] below
    the diagonal."""
    M = len(v)
    k = np.arange(M)[:, None]
    j = np.arange(M)[None, :]
    r = k - j
    return np.where(r >= 0, v[r % M], -v[r % M])


def _circ(v):
    M = len(v)
    k = np.arange(M)[:, None]
    j = np.arange(M)[None, :]
    return v[(k - j) % M]


def _build_weights(decoders: np.ndarray):
    """Decoder-contiguous direct row-block weight buffers (bf16).

    wk1[:, ((k*3+mat)*D + d)*256 :][:256]: k-th 128-row block of the 256x256
    Karatsuba matrix mat in {P, R, P-R} of skew512(h_d)/2.
    wk2[:, (mat*D + d)*128 :][:128]: {P2, R2, -R2} blocks of
    skew256(gm_d)/4.   wk3: {circ128(gpp_d)/8, skew128(gpm_d)/8}.
    """
    dec = np.asarray(decoders, np.float32)
    wk1 = np.empty((128, 2 * 3 * D * 256), np.float32)
    wk2 = np.empty((128, 3 * D * 128), np.float32)
    wk3 = np.empty((128, 2 * D * 128), np.float32)
    for d in range(D):
        v = dec[d]
        g = v[:512] + v[512:]
        h = (v[:512] - v[512:]) / 2
        gm = (g[:256] - g[256:]) / 4
        gp = g[:256] + g[256:]
        gpp = (gp[:128] + gp[128:]) / 8
        gpm = (gp[:128] - gp[128:]) / 8

        S = _skew(h)                      # 512x512
        mats1 = (S[:256, :256], S[256:, :256],
                 S[:256, :256] - S[256:, :256])     # P, R, P-R
        for mat, W in enumerate(mats1):
            for k in range(2):
                c0 = ((k * 3 + mat) * D + d) * 256
                wk1[:, c0:c0 + 256] = W[k * 128:(k + 1) * 128, :]
        S2 = _skew(gm)                    # 256x256
        mats2 = (S2[:128, :128], S2[128:, :128],
                 -S2[128:, :128])                   # P2, R2, -R2
        for mat, W in enumerate(mats2):
            c0 = (mat * D + d) * 128
            wk2[:, c0:c0 + 128] = W
        for which, W in enumerate((_circ(gpp), _skew(gpm))):
            c0 = (which * D + d) * 128
            wk3[:, c0:c0 + 128] = W
    b = ml_dtypes.bfloat16
    return wk1.astype(b), wk2.astype(b), wk3.astype(b)


def _build_in_maps(problemhrr, lemmahrr, decoders):
    b = ml_dtypes.bfloat16
    wk1, wk2, wk3 = _build_weights(decoders)

    def t(a):
        return np.ascontiguousarray(a.T).astype(b)

    in_maps = []
    for c in range(NCORES):
        p = problemhrr[c * BPC:(c + 1) * BPC]
        l = lemmahrr[c * BPC:(c + 1) * BPC]
        X = np.concatenate([p, l], axis=0)          # [2048, 1024] f32
        xm = X[:, :512] - X[:, 512:]
        xp = X[:, :512] + X[:, 512:]
        xpp = xp[:, :256] + xp[:, 256:]
        xpm = xp[:, :256] - xp[:, 256:]
        xm0, xm1 = xm[:, :256], xm[:, 256:]
        in_maps.append({
            "xm0": t(xm0), "xm1": t(xm1), "xms": t(xm0 + xm1),
            "xpm0": t(xpm[:, :128]), "xpm1": t(xpm[:, 128:]),
            "xppp": t(xpp[:, :128] + xpp[:, 128:]),
            "xppm": t(xpp[:, :128] - xpp[:, 128:]),
            "wk1": wk1, "wk2": wk2, "wk3": wk3,
        })
    return in_maps


def _bf16_to_f32(a: np.ndarray) -> np.ndarray:
    return (a.view(np.uint16).astype(np.uint32) << 16).view(np.float32)


def _recombine(dev_out: np.ndarray) -> np.ndarray:
    """[ROWS, DEV_COLS] bf16 phase-major partials -> [ROWS, D*1024] f32."""
    arr = _bf16_to_f32(dev_out)
    m1 = arr[:, 0:4096].reshape(ROWS, D, 256)
    m2 = arr[:, 4096:8192].reshape(ROWS, D, 256)
    m3 = arr[:, 8192:12288].reshape(ROWS, D, 256)
    qm_lo = arr[:, 12288:14336].reshape(ROWS, D, 128)
    qm_hi = arr[:, 14336:16384].reshape(ROWS, D, 128)
    q = arr[:, 16384:18432].reshape(ROWS, D, 128)
    s = arr[:, 18432:20480].reshape(ROWS, D, 128)

    y = np.empty((ROWS, D, 1024), np.float32)
    t = y[:, :, 0:512]
    Qp_lo = y[:, :, 0:128]              # scratch inside t
    np.add(q, s, out=Qp_lo)
    Qp_hi = y[:, :, 128:256]
    np.subtract(q, s, out=Qp_hi)
    Qp = y[:, :, 0:256]
    Qm = np.concatenate([qm_lo, qm_hi], axis=-1)            # [.,.,256]
    np.subtract(Qp, Qm, out=y[:, :, 256:512])
    np.add(Qp, Qm, out=Qp)              # t = [Qp+Qm, Qp-Qm]
    Pm = np.concatenate([m1 + m2, m3 - m1 + m2], axis=-1)   # [.,.,512]
    np.subtract(t, Pm, out=y[:, :, 512:1024])
    np.add(t, Pm, out=t)
    return y.reshape(ROWS, D * 1024)


def kernel(problemhrr: np.ndarray, lemmahrr: np.ndarray,
           decoders: np.ndarray) -> np.ndarray:
    from concourse.bass_utils import run_bass_kernel_spmd

    problemhrr = np.asarray(problemhrr, dtype=np.float32)
    lemmahrr = np.asarray(lemmahrr, dtype=np.float32)
    decoders = np.asarray(decoders, dtype=np.float32)

    nc = _get_program()
    in_maps = _build_in_maps(problemhrr, lemmahrr, decoders)
    res = run_bass_kernel_spmd(nc, in_maps, list(range(NCORES)))

    full = np.empty((B, OUT_COLS), np.float32)
    full[:, :HRR] = problemhrr
    full[:, HRR:2 * HRR] = lemmahrr
    for c in range(NCORES):
        y = _recombine(res.results[c]["out"])
        rows = slice(c * BPC, (c + 1) * BPC)
        full[rows, 2 * HRR:2 * HRR + DN] = y[:BPC]
        full[rows, 2 * HRR + DN:] = y[BPC:]
    return full


# revision 35
# speedup vs baseline: 1.6034x; 1.3055x over previous
"""Trainium2 kernel for nn_Decoder_featurizer: HRR decoder featurization.

reference: out = concat([p, l, assoc(dec_d, p)..., assoc(dec_d, l)...], -1)
where assoc(d, x)[j] = sum_t d[t] * x[(j+t) % N]  (circular correlation).

Circular correlation is a circulant matmul y = x @ C_d with
C_d[k, j] = dec[d, (k-j) % N].  v5 cuts the PE work 3.5x below the dense
circulant by composing two matrix identities, recursively:

 (1) CRT split of a cyclic ring:  x^2M-1 = (x^M-1)(x^M+1) turns a
     circulant-2M matmul into a circulant-M plus a negacyclic-M matmul on
     folded inputs (2 half-size mults instead of 4 quarters).
 (2) Karatsuba on the negacyclic ring: the skew-circulant has block form
     [[P, -R], [R, P]] (complex-multiplication structure), so it costs 3
     half-size Toeplitz matmuls (m1=x0@P, m2=x1@R, m3=(x0+x1)@(P-R)) with
     the butterfly  y = [m1+m2, m3-m1+m2].

Applied to N=1024:  cyc1024 -> cyc512 + neg512;  cyc512 -> cyc256 + neg256;
cyc256 -> cyc128 + neg128;  neg512 -> 3 x Toep256 (Karatsuba).  Leaves per
decoder:
  m1,m2,m3     : 3 matmuls [B,256]@[256,256]   (neg512 Karatsuba)
  Qm_lo,Qm_hi  : 4 matmuls [B,128]@[128,128]   (neg256 direct block form,
                 pairs PSUM-accumulated: Qm_lo = x0@P2 + x1@R2,
                 Qm_hi = x1@P2 - x0@R2 -- trades 1 extra small matmul for
                 shipping 256 instead of 384 partial cols; the drain/DMA
                 path, not the PE, is the binding budget)
  q            : [B,128]@circ128               (cyc128)
  s            : [B,128]@skew128               (neg128)
= 245760 MACs/row/decoder vs 1048576 dense.  All folded x operands are
host-precomputed (shipped transposed, bf16); the remaining butterfly
recombines run on the host in f32.  The passthrough columns never touch
the device.

Every device matmul has N=512: weights are stored decoder-contiguous so one
moving operand covers 2 decoders (256-wide leaves) or 4 decoders (128-wide
leaves) -- small-N matmuls measurably pay a large per-MM floor (v4 post-
mortem).  v7 organizes the work as single-leaf PSUM phases spanning 8
decoders (256-wide leaves, 4 pairs) or all 16 (128-wide leaves), so each
stationary LDWEIGHTS feeds 4 N=512 matmuls.  Per m-tile: 10 phases of 4
PSUM banks rotating through 2 pool buffers; DVE and ScalarE split each
phase's psum->SBUF bf16 drain so banks free within the next phase's PE
window; all drains land in one stage tile and a single 5.2 MB DMA per
m-tile writes out at ~97%% DMA efficiency.  Device output is raw partials
in bf16, phase-major.  Redundant LDWEIGHTS are deduped at the BIR level.
"""

import numpy as np
import ml_dtypes

HRR = 1024
D = 16
B = 8192
NCORES = 8
BPC = B // NCORES            # batch rows per core
ROWS = 2 * BPC               # rows per core (problem + lemma stacked)
DN = D * HRR                 # 16384 assoc features per input
OUT_COLS = 2 * HRR + 2 * DN  # 34816
DEV_COLS = 20480             # device partial columns per row
# phase-major column layout: m1,m2,m3 [16 dec x 256], Qm_lo,Qm_hi,q,s [16x128]

_CACHE = {}


def _build_program(loop_iters: int = 1, pe_only: bool = False,
                   no_dma: bool = False):
    """pe_only=True builds a timing-diagnostic variant with the drain
    copies and output DMAs removed from the loop (output stays zero);
    no_dma=True keeps the drains but skips the output DMAs."""
    import contextlib
    import concourse.bacc as bacc
    import concourse.mybir as mybir
    from concourse.tile import TileContext

    nc = bacc.Bacc("TRN2", target_bir_lowering=False, debug=False,
                   num_devices=NCORES)
    bf16 = mybir.dt.bfloat16
    xin = {}
    for name in ("xm0", "xm1", "xms"):
        xin[name] = nc.dram_tensor(name, [256, ROWS], bf16,
                                   kind="ExternalInput").ap()
    for name in ("xpm0", "xpm1", "xppp", "xppm"):
        xin[name] = nc.dram_tensor(name, [128, ROWS], bf16,
                                   kind="ExternalInput").ap()
    wk1 = nc.dram_tensor("wk1", [128, 2 * 3 * D * 256], bf16,
                         kind="ExternalInput").ap()
    wk2 = nc.dram_tensor("wk2", [128, 3 * D * 128], bf16,
                         kind="ExternalInput").ap()
    wk3 = nc.dram_tensor("wk3", [128, 2 * D * 128], bf16,
                         kind="ExternalInput").ap()
    out = nc.dram_tensor("out", [ROWS, DEV_COLS], bf16,
                         kind="ExternalOutput").ap()

    with TileContext(nc) as tc:
        with (
            tc.tile_pool(name="xp", bufs=1) as xpool,
            tc.tile_pool(name="wp", bufs=1) as wpool,
            tc.tile_pool(name="ps", bufs=2, space="PSUM") as pspool,
            tc.tile_pool(name="ob", bufs=2) as opool,
        ):
            # resident transposed folded activations
            xt = {}
            for name in ("xm0", "xm1", "xms"):
                tiles = []
                for k in range(2):
                    t = xpool.tile([128, ROWS], bf16, tag=f"{name}_{k}")
                    nc.sync.dma_start(
                        out=t[:], in_=xin[name][k * 128:(k + 1) * 128, :])
                    tiles.append(t)
                xt[name] = tiles
            for name in ("xpm0", "xpm1", "xppp", "xppm"):
                t = xpool.tile([128, ROWS], bf16, tag=name)
                nc.sync.dma_start(out=t[:], in_=xin[name][:, :])
                xt[name] = t

            # resident weights (decoder-contiguous direct row blocks)
            wk1t = wpool.tile([128, 2 * 3 * D * 256], bf16, tag="wk1")
            nc.sync.dma_start(out=wk1t[:], in_=wk1[:, :])
            wk2t = wpool.tile([128, 3 * D * 128], bf16, tag="wk2")
            nc.sync.dma_start(out=wk2t[:], in_=wk2[:, :])
            wk3t = wpool.tile([128, 2 * D * 128], bf16, tag="wk3")
            nc.sync.dma_start(out=wk3t[:], in_=wk3[:, :])

            loop_cm = (tc.For_i(0, loop_iters, 1,
                                hint_engines=(mybir.EngineType.PE,
                                              mybir.EngineType.SP,
                                              mybir.EngineType.DVE,
                                              mybir.EngineType.Activation))
                       if loop_iters > 1 else contextlib.nullcontext())
            with loop_cm:
              for m in range(16):
                ms = slice(m * 128, (m + 1) * 128)
                # all 10 phase drains land in one stage tile; a single
                # 5.2 MB DMA per m-tile runs at ~97% DMA efficiency
                stage = (None if pe_only else
                         opool.tile([128, DEV_COLS], bf16, name="stage"))
                st = {"fill": 0, "flip": 0}

                def drain_dma(ps, width):
                    """One full-width drain copy per phase, alternating
                    DVE/ACT (fewer ops -> less per-op sync overhead)."""
                    f0 = st["fill"]
                    st["fill"] += width
                    if pe_only:
                        return
                    engs = (nc.vector.tensor_copy, nc.scalar.copy)
                    f = st["flip"]
                    st["flip"] ^= 1
                    engs[f](out=stage[:, f0:f0 + width], in_=ps[:, 0:width])

                # m-leaves (K=256): one phase per Karatsuba matrix per
                # 8-decoder half; each LDWEIGHTS feeds 4 N=512 matmuls.
                xmk = (xt["xm0"], xt["xm1"], xt["xms"])
                for mat in range(3):
                    for h in range(2):
                        ps = pspool.tile([128, 2048], mybir.dt.float32,
                                         name="ps")
                        for k in range(2):
                            lhsT = xmk[mat][k][:, ms]
                            for p in range(4):
                                w0 = ((k * 3 + mat) * D + 8 * h
                                      + 2 * p) * 256
                                nc.tensor.matmul(
                                    ps[:, p * 512:(p + 1) * 512], lhsT,
                                    wk1t[:, w0:w0 + 512],
                                    start=(k == 0), stop=(k == 1))
                        drain_dma(ps, 2048)

                # neg256 direct block form, PSUM-accumulated (K=128 each):
                #   Qm_lo = xpm0 @ P2 + xpm1 @ R2
                #   Qm_hi = xpm1 @ P2 - xpm0 @ R2
                # (wk2 stores [P2 | R2 | -R2] 16-decoder blocks)
                for ops in (((xt["xpm0"], 0), (xt["xpm1"], 1)),
                            ((xt["xpm1"], 0), (xt["xpm0"], 2))):
                    ps = pspool.tile([128, 2048], mybir.dt.float32,
                                     name="ps")
                    for step, (lx, mat) in enumerate(ops):
                        lhsT = lx[:, ms]
                        for p in range(4):
                            w0 = (mat * D + 4 * p) * 128
                            nc.tensor.matmul(
                                ps[:, p * 512:(p + 1) * 512], lhsT,
                                wk2t[:, w0:w0 + 512],
                                start=(step == 0), stop=(step == 1))
                    drain_dma(ps, 2048)

                # cyc128/neg128 leaves (K=128): one phase per leaf spanning
                # all 16 decoders; one LDWEIGHTS feeds 4 N=512 matmuls.
                for lx, mat in ((xt["xppp"], 0), (xt["xppm"], 1)):
                    ps = pspool.tile([128, 2048], mybir.dt.float32,
                                     name="ps")
                    lhsT = lx[:, ms]
                    for p in range(4):
                        w0 = (mat * D + 4 * p) * 128
                        nc.tensor.matmul(
                            ps[:, p * 512:(p + 1) * 512], lhsT,
                            wk3t[:, w0:w0 + 512], start=True, stop=True)
                    drain_dma(ps, 2048)
                if not pe_only and not no_dma:
                    nc.sync.dma_start(out=out[ms, :], in_=stage[:])
    _finalize_with_dedup(nc)
    return nc


def _dedup_ldweights(nc):
    """Drop redundant InstLdweights from the PE stream.

    bacc emits every matmul as an (InstLdweights, InstMatmult) pair; the
    matmult is non-self-loading, so the PE weight register persists across
    matmuls.  Consecutive pairs with an identical stationary AP reload the
    same weights (~107ns each on HW).  Drop an InstLdweights when its
    signature matches the previous one on the PE stream AND it carries no
    waits/updates.  Conservatively resets tracking at block boundaries and
    on any other PE instruction.
    """
    import concourse.mybir as mybir

    InstLdweights = mybir.InstLdweights
    InstMatmult = mybir.InstMatmult
    n_drop = 0
    for fn in nc.m.functions:
        for blk in fn.blocks:
            keep = []
            last_sig = None
            for inst in blk.instructions:
                if isinstance(inst, InstLdweights):
                    pap = inst.ins[0]
                    sig = (pap.memref, pap.offset, str(pap.ap),
                           str(pap.dtype),
                           str(getattr(inst, "perf_mode", None)),
                           str(getattr(inst, "is_transpose", None)),
                           str(getattr(inst, "tile_position", None)))
                    si = inst.sync_info
                    bare = si is None or (len(si.on_wait) == 0
                                          and len(si.on_update) == 0)
                    if sig == last_sig and bare:
                        n_drop += 1
                        continue
                    last_sig = sig
                elif getattr(inst, "engine", None) == mybir.EngineType.PE:
                    if isinstance(inst, InstMatmult):
                        if getattr(inst, "is_transpose", None):
                            last_sig = None
                    else:
                        last_sig = None
                keep.append(inst)
            if n_drop:
                try:
                    blk.instructions = keep
                except Exception:
                    insts = blk.instructions
                    while len(insts):
                        insts.pop()
                    for i in keep:
                        insts.append(i)
    return n_drop


def _finalize_with_dedup(nc):
    orig_mv = nc.move_matmul_waits_to_ldweights

    def _mv():
        orig_mv()
        _dedup_ldweights(nc)

    nc.move_matmul_waits_to_ldweights = _mv
    nc.finalize()


def _get_program(loop_iters: int = 1):
    key = f"nc{loop_iters}"
    if key not in _CACHE:
        _CACHE[key] = _build_program(loop_iters)
    return _CACHE[key]


def _skew(v):
    """Skew-circulant (negacyclic) matrix W[k,j] = v[k-j], -v[k-j+M] below
    the diagonal."""
    M = len(v)
    k = np.arange(M)[:, None]
    j = np.arange(M)[None, :]
    r = k - j
    return np.where(r >= 0, v[r % M], -v[r % M])


def _circ(v):
    M = len(v)
    k = np.arange(M)[:, None]
    j = np.arange(M)[None, :]
    return v[(k - j) % M]


def _build_weights(decoders: np.ndarray):
    """Decoder-contiguous direct row-block weight buffers (bf16).

    wk1[:, ((k*3+mat)*D + d)*256 :][:256]: k-th 128-row block of the 256x256
    Karatsuba matrix mat in {P, R, P-R} of skew512(h_d)/2.
    wk2[:, (mat*D + d)*128 :][:128]: {P2, R2, -R2} blocks of
    skew256(gm_d)/4.   wk3: {circ128(gpp_d)/8, skew128(gpm_d)/8}.
    """
    dec = np.asarray(decoders, np.float32)
    wk1 = np.empty((128, 2 * 3 * D * 256), np.float32)
    wk2 = np.empty((128, 3 * D * 128), np.float32)
    wk3 = np.empty((128, 2 * D * 128), np.float32)
    for d in range(D):
        v = dec[d]
        g = v[:512] + v[512:]
        h = (v[:512] - v[512:]) / 2
        gm = (g[:256] - g[256:]) / 4
        gp = g[:256] + g[256:]
        gpp = (gp[:128] + gp[128:]) / 8
        gpm = (gp[:128] - gp[128:]) / 8

        S = _skew(h)                      # 512x512
        mats1 = (S[:256, :256], S[256:, :256],
                 S[:256, :256] - S[256:, :256])     # P, R, P-R
        for mat, W in enumerate(mats1):
            for k in range(2):
                c0 = ((k * 3 + mat) * D + d) * 256
                wk1[:, c0:c0 + 256] = W[k * 128:(k + 1) * 128, :]
        S2 = _skew(gm)                    # 256x256
        mats2 = (S2[:128, :128], S2[128:, :128],
                 -S2[128:, :128])                   # P2, R2, -R2
        for mat, W in enumerate(mats2):
            c0 = (mat * D + d) * 128
            wk2[:, c0:c0 + 128] = W
        for which, W in enumerate((_circ(gpp), _skew(gpm))):
            c0 = (which * D + d) * 128
            wk3[:, c0:c0 + 128] = W
    b = ml_dtypes.bfloat16
    return wk1.astype(b), wk2.astype(b), wk3.astype(b)


def _build_in_maps(problemhrr, lemmahrr, decoders):
    b = ml_dtypes.bfloat16
    wk1, wk2, wk3 = _build_weights(decoders)

    def t(a):
        return np.ascontiguousarray(a.T).astype(b)

    in_maps = []
    for c in range(NCORES):
        p = problemhrr[c * BPC:(c + 1) * BPC]
        l = lemmahrr[c * BPC:(c + 1) * BPC]
        X = np.concatenate([p, l], axis=0)          # [2048, 1024] f32
        xm = X[:, :512] - X[:, 512:]
        xp = X[:, :512] + X[:, 512:]
        xpp = xp[:, :256] + xp[:, 256:]
        xpm = xp[:, :256] - xp[:, 256:]
        xm0, xm1 = xm[:, :256], xm[:, 256:]
        in_maps.append({
            "xm0": t(xm0), "xm1": t(xm1), "xms": t(xm0 + xm1),
            "xpm0": t(xpm[:, :128]), "xpm1": t(xpm[:, 128:]),
            "xppp": t(xpp[:, :128] + xpp[:, 128:]),
            "xppm": t(xpp[:, :128] - xpp[:, 128:]),
            "wk1": wk1, "wk2": wk2, "wk3": wk3,
        })
    return in_maps


def _bf16_to_f32(a: np.ndarray) -> np.ndarray:
    return (a.view(np.uint16).astype(np.uint32) << 16).view(np.float32)


def _recombine(dev_out: np.ndarray) -> np.ndarray:
    """[ROWS, DEV_COLS] bf16 phase-major partials -> [ROWS, D*1024] f32."""
    arr = _bf16_to_f32(dev_out)
    m1 = arr[:, 0:4096].reshape(ROWS, D, 256)
    m2 = arr[:, 4096:8192].reshape(ROWS, D, 256)
    m3 = arr[:, 8192:12288].reshape(ROWS, D, 256)
    qm_lo = arr[:, 12288:14336].reshape(ROWS, D, 128)
    qm_hi = arr[:, 14336:16384].reshape(ROWS, D, 128)
    q = arr[:, 16384:18432].reshape(ROWS, D, 128)
    s = arr[:, 18432:20480].reshape(ROWS, D, 128)

    y = np.empty((ROWS, D, 1024), np.float32)
    t = y[:, :, 0:512]
    Qp_lo = y[:, :, 0:128]              # scratch inside t
    np.add(q, s, out=Qp_lo)
    Qp_hi = y[:, :, 128:256]
    np.subtract(q, s, out=Qp_hi)
    Qp = y[:, :, 0:256]
    Qm = np.concatenate([qm_lo, qm_hi], axis=-1)            # [.,.,256]
    np.subtract(Qp, Qm, out=y[:, :, 256:512])
    np.add(Qp, Qm, out=Qp)              # t = [Qp+Qm, Qp-Qm]
    Pm = np.concatenate([m1 + m2, m3 - m1 + m2], axis=-1)   # [.,.,512]
    np.subtract(t, Pm, out=y[:, :, 512:1024])
    np.add(t, Pm, out=t)
    return y.reshape(ROWS, D * 1024)


def kernel(problemhrr: np.ndarray, lemmahrr: np.ndarray,
           decoders: np.ndarray) -> np.ndarray:
    from concourse.bass_utils import run_bass_kernel_spmd

    problemhrr = np.asarray(problemhrr, dtype=np.float32)
    lemmahrr = np.asarray(lemmahrr, dtype=np.float32)
    decoders = np.asarray(decoders, dtype=np.float32)

    nc = _get_program()
    in_maps = _build_in_maps(problemhrr, lemmahrr, decoders)
    res = run_bass_kernel_spmd(nc, in_maps, list(range(NCORES)))

    full = np.empty((B, OUT_COLS), np.float32)
    full[:, :HRR] = problemhrr
    full[:, HRR:2 * HRR] = lemmahrr
    for c in range(NCORES):
        y = _recombine(res.results[c]["out"])
        rows = slice(c * BPC, (c + 1) * BPC)
        full[rows, 2 * HRR:2 * HRR + DN] = y[:BPC]
        full[rows, 2 * HRR + DN:] = y[BPC:]
    return full


# revision 38
# speedup vs baseline: 1.6284x; 1.0156x over previous
"""Trainium2 kernel for nn_Decoder_featurizer: HRR decoder featurization.

reference: out = concat([p, l, assoc(dec_d, p)..., assoc(dec_d, l)...], -1)
where assoc(d, x)[j] = sum_t d[t] * x[(j+t) % N]  (circular correlation).

Circular correlation is a circulant matmul y = x @ C_d with
C_d[k, j] = dec[d, (k-j) % N].  v5 cuts the PE work 3.5x below the dense
circulant by composing two matrix identities, recursively:

 (1) CRT split of a cyclic ring:  x^2M-1 = (x^M-1)(x^M+1) turns a
     circulant-2M matmul into a circulant-M plus a negacyclic-M matmul on
     folded inputs (2 half-size mults instead of 4 quarters).
 (2) Karatsuba on the negacyclic ring: the skew-circulant has block form
     [[P, -R], [R, P]] (complex-multiplication structure), so it costs 3
     half-size Toeplitz matmuls (m1=x0@P, m2=x1@R, m3=(x0+x1)@(P-R)) with
     the butterfly  y = [m1+m2, m3-m1+m2].

Applied to N=1024:  cyc1024 -> cyc512 + neg512;  cyc512 -> cyc256 + neg256;
cyc256 -> cyc128 + neg128;  neg512 -> 3 x Toep256 (Karatsuba).  Leaves per
decoder:
  m1,m2,m3     : 3 matmuls [B,256]@[256,256]   (neg512 Karatsuba)
  Qm_lo,Qm_hi  : 4 matmuls [B,128]@[128,128]   (neg256 direct block form,
                 pairs PSUM-accumulated: Qm_lo = x0@P2 + x1@R2,
                 Qm_hi = x1@P2 - x0@R2 -- trades 1 extra small matmul for
                 shipping 256 instead of 384 partial cols; the drain/DMA
                 path, not the PE, is the binding budget)
  q            : [B,128]@circ128               (cyc128)
  s            : [B,128]@skew128               (neg128)
= 245760 MACs/row/decoder vs 1048576 dense.  All folded x operands are
host-precomputed (shipped transposed, bf16); the remaining butterfly
recombines run on the host in f32.  The passthrough columns never touch
the device.

Every device matmul has N=512: weights are stored decoder-contiguous so one
moving operand covers 2 decoders (256-wide leaves) or 4 decoders (128-wide
leaves) -- small-N matmuls measurably pay a large per-MM floor (v4 post-
mortem).  v7 organizes the work as single-leaf PSUM phases spanning 8
decoders (256-wide leaves, 4 pairs) or all 16 (128-wide leaves), so each
stationary LDWEIGHTS feeds 4 N=512 matmuls.  Per m-tile: 10 phases of 4
PSUM banks rotating through 2 pool buffers; DVE and ScalarE split each
phase's psum->SBUF bf16 drain so banks free within the next phase's PE
window; all drains land in one stage tile and a single 5.2 MB DMA per
m-tile writes out at ~97%% DMA efficiency.  Device output is raw partials
in bf16, phase-major.  Redundant LDWEIGHTS are deduped at the BIR level.
"""

import numpy as np
import ml_dtypes

HRR = 1024
D = 16
B = 8192
NCORES = 8
BPC = B // NCORES            # batch rows per core
ROWS = 2 * BPC               # rows per core (problem + lemma stacked)
DN = D * HRR                 # 16384 assoc features per input
OUT_COLS = 2 * HRR + 2 * DN  # 34816
DEV_COLS = 20480             # device partial columns per row
# phase-major column layout: m1,m2,m3 [16 dec x 256], Qm_lo,Qm_hi,q,s [16x128]

_CACHE = {}


def _build_program(loop_iters: int = 1, pe_only: bool = False,
                   no_dma: bool = False):
    """pe_only=True builds a timing-diagnostic variant with the drain
    copies and output DMAs removed from the loop (output stays zero);
    no_dma=True keeps the drains but skips the output DMAs."""
    import contextlib
    import concourse.bacc as bacc
    import concourse.mybir as mybir
    from concourse.tile import TileContext

    nc = bacc.Bacc("TRN2", target_bir_lowering=False, debug=False,
                   num_devices=NCORES)
    bf16 = mybir.dt.bfloat16
    xin = {}
    for name in ("xm0", "xm1", "xms"):
        xin[name] = nc.dram_tensor(name, [256, ROWS], bf16,
                                   kind="ExternalInput").ap()
    for name in ("xpm0", "xpm1", "xppp", "xppm"):
        xin[name] = nc.dram_tensor(name, [128, ROWS], bf16,
                                   kind="ExternalInput").ap()
    wk1 = nc.dram_tensor("wk1", [128, 2 * 3 * D * 256], bf16,
                         kind="ExternalInput").ap()
    wk2 = nc.dram_tensor("wk2", [128, 3 * D * 128], bf16,
                         kind="ExternalInput").ap()
    wk3 = nc.dram_tensor("wk3", [128, 2 * D * 128], bf16,
                         kind="ExternalInput").ap()
    out = nc.dram_tensor("out", [ROWS, DEV_COLS], bf16,
                         kind="ExternalOutput").ap()

    with TileContext(nc) as tc:
        with (
            tc.tile_pool(name="xp", bufs=1) as xpool,
            tc.tile_pool(name="wp", bufs=1) as wpool,
            tc.tile_pool(name="ps", bufs=2, space="PSUM") as pspool,
            tc.tile_pool(name="ob", bufs=4) as opool,
        ):
            # resident transposed folded activations
            xt = {}
            for name in ("xm0", "xm1", "xms"):
                tiles = []
                for k in range(2):
                    t = xpool.tile([128, ROWS], bf16, tag=f"{name}_{k}")
                    nc.sync.dma_start(
                        out=t[:], in_=xin[name][k * 128:(k + 1) * 128, :])
                    tiles.append(t)
                xt[name] = tiles
            for name in ("xpm0", "xpm1", "xppp", "xppm"):
                t = xpool.tile([128, ROWS], bf16, tag=name)
                nc.sync.dma_start(out=t[:], in_=xin[name][:, :])
                xt[name] = t

            # resident weights (decoder-contiguous direct row blocks)
            wk1t = wpool.tile([128, 2 * 3 * D * 256], bf16, tag="wk1")
            nc.sync.dma_start(out=wk1t[:], in_=wk1[:, :])
            wk2t = wpool.tile([128, 3 * D * 128], bf16, tag="wk2")
            nc.sync.dma_start(out=wk2t[:], in_=wk2[:, :])
            wk3t = wpool.tile([128, 2 * D * 128], bf16, tag="wk3")
            nc.sync.dma_start(out=wk3t[:], in_=wk3[:, :])

            loop_cm = (tc.For_i(0, loop_iters, 1,
                                hint_engines=(mybir.EngineType.PE,
                                              mybir.EngineType.SP,
                                              mybir.EngineType.DVE,
                                              mybir.EngineType.Activation))
                       if loop_iters > 1 else contextlib.nullcontext())
            with loop_cm:
              for m in range(16):
                ms = slice(m * 128, (m + 1) * 128)
                # phase drains accumulate into half-m stage tiles (bufs=4)
                # flushed as 2.6 MB DMAs -- finer DMA pipelining, same SBUF
                st = {"tile": None, "fill": 0, "col": 0, "flip": 0}

                def drain_dma(ps, width):
                    """One full-width drain copy per phase, alternating
                    DVE/ACT (fewer ops -> less per-op sync overhead)."""
                    if pe_only:
                        return
                    if st["tile"] is None:
                        st["tile"] = opool.tile([128, DEV_COLS // 2], bf16,
                                                name="stage")
                        st["fill"] = 0
                    stage, f0 = st["tile"], st["fill"]
                    engs = (nc.vector.tensor_copy, nc.scalar.copy)
                    f = st["flip"]
                    st["flip"] ^= 1
                    engs[f](out=stage[:, f0:f0 + width], in_=ps[:, 0:width])
                    st["fill"] += width
                    if st["fill"] >= DEV_COLS // 2:
                        if not no_dma:
                            nc.sync.dma_start(
                                out=out[ms,
                                        st["col"]:st["col"] + st["fill"]],
                                in_=stage[:, 0:st["fill"]])
                        st["col"] += st["fill"]
                        st["tile"] = None

                # m-leaves (K=256): one phase per Karatsuba matrix per
                # 8-decoder half; each LDWEIGHTS feeds 4 N=512 matmuls.
                xmk = (xt["xm0"], xt["xm1"], xt["xms"])
                for mat in range(3):
                    for h in range(2):
                        ps = pspool.tile([128, 2048], mybir.dt.float32,
                                         name="ps")
                        for k in range(2):
                            lhsT = xmk[mat][k][:, ms]
                            for p in range(4):
                                w0 = ((k * 3 + mat) * D + 8 * h
                                      + 2 * p) * 256
                                nc.tensor.matmul(
                                    ps[:, p * 512:(p + 1) * 512], lhsT,
                                    wk1t[:, w0:w0 + 512],
                                    start=(k == 0), stop=(k == 1))
                        drain_dma(ps, 2048)

                # neg256 direct block form, PSUM-accumulated (K=128 each):
                #   Qm_lo = xpm0 @ P2 + xpm1 @ R2
                #   Qm_hi = xpm1 @ P2 - xpm0 @ R2
                # (wk2 stores [P2 | R2 | -R2] 16-decoder blocks)
                for ops in (((xt["xpm0"], 0), (xt["xpm1"], 1)),
                            ((xt["xpm1"], 0), (xt["xpm0"], 2))):
                    ps = pspool.tile([128, 2048], mybir.dt.float32,
                                     name="ps")
                    for step, (lx, mat) in enumerate(ops):
                        lhsT = lx[:, ms]
                        for p in range(4):
                            w0 = (mat * D + 4 * p) * 128
                            nc.tensor.matmul(
                                ps[:, p * 512:(p + 1) * 512], lhsT,
                                wk2t[:, w0:w0 + 512],
                                start=(step == 0), stop=(step == 1))
                    drain_dma(ps, 2048)

                # cyc128/neg128 leaves (K=128): one phase per leaf spanning
                # all 16 decoders; one LDWEIGHTS feeds 4 N=512 matmuls.
                for lx, mat in ((xt["xppp"], 0), (xt["xppm"], 1)):
                    ps = pspool.tile([128, 2048], mybir.dt.float32,
                                     name="ps")
                    lhsT = lx[:, ms]
                    for p in range(4):
                        w0 = (mat * D + 4 * p) * 128
                        nc.tensor.matmul(
                            ps[:, p * 512:(p + 1) * 512], lhsT,
                            wk3t[:, w0:w0 + 512], start=True, stop=True)
                    drain_dma(ps, 2048)
    _finalize_with_dedup(nc)
    return nc


def _dedup_ldweights(nc):
    """Drop redundant InstLdweights from the PE stream.

    bacc emits every matmul as an (InstLdweights, InstMatmult) pair; the
    matmult is non-self-loading, so the PE weight register persists across
    matmuls.  Consecutive pairs with an identical stationary AP reload the
    same weights (~107ns each on HW).  Drop an InstLdweights when its
    signature matches the previous one on the PE stream AND it carries no
    waits/updates.  Conservatively resets tracking at block boundaries and
    on any other PE instruction.
    """
    import concourse.mybir as mybir

    InstLdweights = mybir.InstLdweights
    InstMatmult = mybir.InstMatmult
    n_drop = 0
    for fn in nc.m.functions:
        for blk in fn.blocks:
            keep = []
            last_sig = None
            for inst in blk.instructions:
                if isinstance(inst, InstLdweights):
                    pap = inst.ins[0]
                    sig = (pap.memref, pap.offset, str(pap.ap),
                           str(pap.dtype),
                           str(getattr(inst, "perf_mode", None)),
                           str(getattr(inst, "is_transpose", None)),
                           str(getattr(inst, "tile_position", None)))
                    si = inst.sync_info
                    bare = si is None or (len(si.on_wait) == 0
                                          and len(si.on_update) == 0)
                    if sig == last_sig and bare:
                        n_drop += 1
                        continue
                    last_sig = sig
                elif getattr(inst, "engine", None) == mybir.EngineType.PE:
                    if isinstance(inst, InstMatmult):
                        if getattr(inst, "is_transpose", None):
                            last_sig = None
                    else:
                        last_sig = None
                keep.append(inst)
            if n_drop:
                try:
                    blk.instructions = keep
                except Exception:
                    insts = blk.instructions
                    while len(insts):
                        insts.pop()
                    for i in keep:
                        insts.append(i)
    return n_drop


def _finalize_with_dedup(nc):
    orig_mv = nc.move_matmul_waits_to_ldweights

    def _mv():
        orig_mv()
        _dedup_ldweights(nc)

    nc.move_matmul_waits_to_ldweights = _mv
    nc.finalize()


def _get_program(loop_iters: int = 1):
    key = f"nc{loop_iters}"
    if key not in _CACHE:
        _CACHE[key] = _build_program(loop_iters)
    return _CACHE[key]


def _skew(v):
    """Skew-circulant (negacyclic) matrix W[k,j] = v[k-j], -v[k-j+M] below
    the diagonal."""
    M = len(v)
    k = np.arange(M)[:, None]
    j = np.arange(M)[None, :]
    r = k - j
    return np.where(r >= 0, v[r % M], -v[r % M])


def _circ(v):
    M = len(v)
    k = np.arange(M)[:, None]
    j = np.arange(M)[None, :]
    return v[(k - j) % M]


def _build_weights(decoders: np.ndarray):
    """Decoder-contiguous direct row-block weight buffers (bf16).

    wk1[:, ((k*3+mat)*D + d)*256 :][:256]: k-th 128-row block of the 256x256
    Karatsuba matrix mat in {P, R, P-R} of skew512(h_d)/2.
    wk2[:, (mat*D + d)*128 :][:128]: {P2, R2, -R2} blocks of
    skew256(gm_d)/4.   wk3: {circ128(gpp_d)/8, skew128(gpm_d)/8}.
    """
    dec = np.asarray(decoders, np.float32)
    wk1 = np.empty((128, 2 * 3 * D * 256), np.float32)
    wk2 = np.empty((128, 3 * D * 128), np.float32)
    wk3 = np.empty((128, 2 * D * 128), np.float32)
    for d in range(D):
        v = dec[d]
        g = v[:512] + v[512:]
        h = (v[:512] - v[512:]) / 2
        gm = (g[:256] - g[256:]) / 4
        gp = g[:256] + g[256:]
        gpp = (gp[:128] + gp[128:]) / 8
        gpm = (gp[:128] - gp[128:]) / 8

        S = _skew(h)                      # 512x512
        mats1 = (S[:256, :256], S[256:, :256],
                 S[:256, :256] - S[256:, :256])     # P, R, P-R
        for mat, W in enumerate(mats1):
            for k in range(2):
                c0 = ((k * 3 + mat) * D + d) * 256
                wk1[:, c0:c0 + 256] = W[k * 128:(k + 1) * 128, :]
        S2 = _skew(gm)                    # 256x256
        mats2 = (S2[:128, :128], S2[128:, :128],
                 -S2[128:, :128])                   # P2, R2, -R2
        for mat, W in enumerate(mats2):
            c0 = (mat * D + d) * 128
            wk2[:, c0:c0 + 128] = W
        for which, W in enumerate((_circ(gpp), _skew(gpm))):
            c0 = (which * D + d) * 128
            wk3[:, c0:c0 + 128] = W
    b = ml_dtypes.bfloat16
    return wk1.astype(b), wk2.astype(b), wk3.astype(b)


def _build_in_maps(problemhrr, lemmahrr, decoders):
    b = ml_dtypes.bfloat16
    wk1, wk2, wk3 = _build_weights(decoders)

    def t(a):
        return np.ascontiguousarray(a.T).astype(b)

    in_maps = []
    for c in range(NCORES):
        p = problemhrr[c * BPC:(c + 1) * BPC]
        l = lemmahrr[c * BPC:(c + 1) * BPC]
        X = np.concatenate([p, l], axis=0)          # [2048, 1024] f32
        xm = X[:, :512] - X[:, 512:]
        xp = X[:, :512] + X[:, 512:]
        xpp = xp[:, :256] + xp[:, 256:]
        xpm = xp[:, :256] - xp[:, 256:]
        xm0, xm1 = xm[:, :256], xm[:, 256:]
        in_maps.append({
            "xm0": t(xm0), "xm1": t(xm1), "xms": t(xm0 + xm1),
            "xpm0": t(xpm[:, :128]), "xpm1": t(xpm[:, 128:]),
            "xppp": t(xpp[:, :128] + xpp[:, 128:]),
            "xppm": t(xpp[:, :128] - xpp[:, 128:]),
            "wk1": wk1, "wk2": wk2, "wk3": wk3,
        })
    return in_maps


def _bf16_to_f32(a: np.ndarray) -> np.ndarray:
    return (a.view(np.uint16).astype(np.uint32) << 16).view(np.float32)


def _recombine(dev_out: np.ndarray) -> np.ndarray:
    """[ROWS, DEV_COLS] bf16 phase-major partials -> [ROWS, D*1024] f32."""
    arr = _bf16_to_f32(dev_out)
    m1 = arr[:, 0:4096].reshape(ROWS, D, 256)
    m2 = arr[:, 4096:8192].reshape(ROWS, D, 256)
    m3 = arr[:, 8192:12288].reshape(ROWS, D, 256)
    qm_lo = arr[:, 12288:14336].reshape(ROWS, D, 128)
    qm_hi = arr[:, 14336:16384].reshape(ROWS, D, 128)
    q = arr[:, 16384:18432].reshape(ROWS, D, 128)
    s = arr[:, 18432:20480].reshape(ROWS, D, 128)

    y = np.empty((ROWS, D, 1024), np.float32)
    t = y[:, :, 0:512]
    Qp_lo = y[:, :, 0:128]              # scratch inside t
    np.add(q, s, out=Qp_lo)
    Qp_hi = y[:, :, 128:256]
    np.subtract(q, s, out=Qp_hi)
    Qp = y[:, :, 0:256]
    Qm = np.concatenate([qm_lo, qm_hi], axis=-1)            # [.,.,256]
    np.subtract(Qp, Qm, out=y[:, :, 256:512])
    np.add(Qp, Qm, out=Qp)              # t = [Qp+Qm, Qp-Qm]
    Pm = np.concatenate([m1 + m2, m3 - m1 + m2], axis=-1)   # [.,.,512]
    np.subtract(t, Pm, out=y[:, :, 512:1024])
    np.add(t, Pm, out=t)
    return y.reshape(ROWS, D * 1024)


def kernel(problemhrr: np.ndarray, lemmahrr: np.ndarray,
           decoders: np.ndarray) -> np.ndarray:
    from concourse.bass_utils import run_bass_kernel_spmd

    problemhrr = np.asarray(problemhrr, dtype=np.float32)
    lemmahrr = np.asarray(lemmahrr, dtype=np.float32)
    decoders = np.asarray(decoders, dtype=np.float32)

    nc = _get_program()
    in_maps = _build_in_maps(problemhrr, lemmahrr, decoders)
    res = run_bass_kernel_spmd(nc, in_maps, list(range(NCORES)))

    full = np.empty((B, OUT_COLS), np.float32)
    full[:, :HRR] = problemhrr
    full[:, HRR:2 * HRR] = lemmahrr
    for c in range(NCORES):
        y = _recombine(res.results[c]["out"])
        rows = slice(c * BPC, (c + 1) * BPC)
        full[rows, 2 * HRR:2 * HRR + DN] = y[:BPC]
        full[rows, 2 * HRR + DN:] = y[BPC:]
    return full
